# revision 1
# baseline (speedup 1.0000x reference)
"""Trainium2 Bass kernel for nn_Attention (GQA + RoPE + sliding-window mask).

Sharding: tensor-parallel over heads across 8 cores. Each core gets 4 q heads
and exactly 1 kv head (32 q / 8 kv heads, GQA group = 4). The reference's
quirky output flatten ((H,S,D)->(H,D,S)->reshape(S, H*D)) makes the final
projection contract over (d-parity, sequence) instead of heads, so the final
output is row-sharded by head block: core c produces rows [256c, 256c+256) of
the (2048, 4096) result with NO collective at all.

Per-core pipeline (all on one NeuronCore, same program on all 8 = pure SPMD):
  phase 1: QKV projections (bf16 matmuls) + RoPE (sqrt(scale) folded into the
           rope tables of both q and k) + DMA transposes into [d, s] layouts.
  phase 2: TRANSPOSED attention. Scores are computed as S^T[k, q] directly
           (K^T tile stationary, Q^T moving), so the exp'd probabilities land
           in SBUF already in the [k, q] layout PV needs - no P transposes.
           Softmax uses no running max (logits are O(10), exp biased by -8
           stays in range); denominators are per-q partition sums computed
           with free 1-wide ones-matmuls on the PE; causal masking is a 0/1
           triangular multiply on the bf16 P tile (DVE). PV then produces
           A[q, d] directly, normalized into the Aall layout by ACT.
  phase 3: final projection vs full wo (bf16), row slice out.
"""

import numpy as np
from contextlib import ExitStack

P = 128
D = 128  # head dim
NH = 4   # q heads per core
CORES = 8
NEG_THRESH = -1e8
EXP_BIAS = -8.0  # constant bias inside exp; cancels in normalization


def build_attention_nc(
    SEQ,
    DIM,
    plan,
    n_uniq,
    n_uniq_add=0,
):
    """Build the per-core Bass program.

    plan: list over q-tiles i (SEQ//128 entries) of lists of (kt, uid, uid_add)
          at 128x128 block granularity. uid == -1: no masking needed.
          uid >= 0: multiply the exp'd P tile by 0/1 block `uid` (DVE).
          uid_add >= 0: add f32 block `uid_add` to scores before exp (general
          additive masks; unused for causal). Blocks absent are fully masked.
    """
    import concourse.bass as bass
    import concourse.bacc as bacc
    import concourse.mybir as mybir
    import concourse.tile as tile

    f32 = mybir.dt.float32
    bf16 = mybir.dt.bfloat16

    ST = SEQ // P          # 16 s-tiles
    DD = DIM // P          # 32 contraction tiles
    EW = NH * D            # 512 q-projection width
    JT = 2 * SEQ // P      # 32 j-tiles for final matmul
    MC = DIM // 512        # 8 output chunks
    ITILES = (NH * 64) // P  # 2 output row tiles
    assert NH == 4 and SEQ % 512 == 0 and DIM % 512 == 0

    nc = bacc.Bacc(trn_type="TRN2", debug=False, num_devices=CORES)

    f8 = mybir.dt.float8e4

    # x and the QKV weights arrive as packed fp8 hi/lo pairs (hi = fp8(v),
    # lo = fp8(v - hi)); three DoubleRow matmuls per contraction-tile pair
    # compute hi*hi + lo*hi + hi*lo at 0.75x the bf16 cycle cost with ~2x
    # BETTER accuracy. Weights are host-scaled by 64 so the lo residuals
    # stay above fp8's subnormal floor; the 1/64 is folded into the rope
    # tables and the V copy.
    # xT[p, st, t, hl, si] = fp8hl(x[st*128+si, t*128+p])
    xT = nc.dram_tensor(
        "xT", [P, ST, DD, 2, P], f8, kind="ExternalInput"
    ).ap()
    # wT[p, t, hl, e] = fp8hl(64 * w_c[e, t*128+p])
    wT = nc.dram_tensor(
        "wT", [P, DD, 2, EW + 2 * D], f8, kind="ExternalInput"
    ).ap()
    cs = nc.dram_tensor("cs", [SEQ, EW], bf16, kind="ExternalInput").ap()
    mb = nc.dram_tensor(
        "maskb", [max(n_uniq, 1), P, P], bf16, kind="ExternalInput"
    ).ap()
    mba = nc.dram_tensor(
        "maskba", [max(n_uniq_add, 1), P, P], f32, kind="ExternalInput"
    ).ap()
    # woT[p, mc, jt, hl, mi] = fp8hl(64 * wo[mc*256+mi, jt*128+p]) -
    # chunk-major so each 256-wide chunk load is one contiguous run per
    # partition (full DMA rate)
    woT = nc.dram_tensor(
        "woT", [P, DIM // 512, JT, 2, 512], f8, kind="ExternalInput"
    ).ap()
    out = nc.dram_tensor("out", [NH * 64, DIM], bf16, kind="ExternalOutput").ap()

    with tile.TileContext(nc) as tc, ExitStack() as ctx:
        const = ctx.enter_context(tc.tile_pool(name="const", bufs=1))
        ones = const.tile([P, 1], bf16)
        nc.vector.memset(ones, 1.0)
        ebias = const.tile([P, 1], f32)
        nc.vector.memset(ebias, EXP_BIAS)
        # touch Exp at t=0 so the ACT table load doesn't stall phase 2
        scr = const.tile([P, 1], f32)
        nc.scalar.activation(
            out=scr, in_=ebias, func=mybir.ActivationFunctionType.Exp
        )
        inv64 = const.tile([P, 1], f32)
        nc.vector.memset(inv64, 1.0 / 64.0)

        pers = ctx.enter_context(tc.tile_pool(name="pers", bufs=1))
        QTt = pers.tile([P, NH, ST * P], bf16)   # [d, h, s]
        KTt = pers.tile([P, ST * P], bf16)       # [d, s]
        Vt = pers.tile([P, ST, D + 4], bf16)     # [k(part), ktile, d | 1 pad]
        # col D is all-ones: PV's rhs [V | 1] also accumulates the softmax
        # denominator into the A psum's column D
        nc.vector.memset(Vt[:, :, D : D + 1], 1.0)
        nc.vector.memset(Vt[:, :, D + 1 : D + 4], 0.0)
        if n_uniq > 0:
            mbt = pers.tile([P, n_uniq, P], bf16)
        if n_uniq_add > 0:
            mbat = pers.tile([P, n_uniq_add, P], f32)

        # ---------------- phase 1: projections + rope + layout ----------------
        with (
            tc.tile_pool(name="wpool", bufs=1) as wpool,
            tc.tile_pool(name="xpool", bufs=6) as xpool,
            tc.tile_pool(name="cspool", bufs=2) as cspool,
            tc.tile_pool(name="rpool", bufs=2) as rpool,
            tc.tile_pool(name="qps", bufs=2, space="PSUM") as qps,
            tc.tile_pool(name="kvps", bufs=2, space="PSUM") as kvps,
        ):
            wTt = wpool.tile([P, DD, 2, EW + 2 * D], f8)
            wTr = wT

            XG = min(8, DD)  # dd-tiles per streamed x chunk
            xTr = xT
            # Fine-grained interleave of the weight loads with s-tile 0's x
            # chunks (both in small pieces) so the first matmuls start within
            # ~2us of kernel start and the pipeline never starves.
            # Weight pieces stream in consumption order (t=0..DD), with
            # s-tile 0's x chunks interleaved among the early pieces.
            st0_x = []
            XG0 = 4
            for g in range(DD // 4):
                nc.sync.dma_start(
                    out=wTt[:, 2 * g : 2 * g + 2, :, :],
                    in_=wTr[:, 2 * g : 2 * g + 2, :, :],
                )
                xTt = xpool.tile([P, XG0, 2, P], f8, tag="xT0")
                nc.sync.dma_start(
                    out=xTt, in_=xTr[:, 0, g * XG0 : (g + 1) * XG0, :, :]
                )
                st0_x.append(xTt)
            # masks are tiny; land them long before phase 2 needs them
            if n_uniq > 0:
                nc.sync.dma_start(out=mbt, in_=mb.rearrange("u p m -> p u m"))
            if n_uniq_add > 0:
                nc.sync.dma_start(out=mbat, in_=mba.rearrange("u p m -> p u m"))

            def stream_x(st):
                chunks = []
                for g in range(DD // XG):
                    xTt = xpool.tile([P, XG, 2, P], f8, tag="xT")
                    nc.sync.dma_start(
                        out=xTt,
                        in_=xTr[:, st, g * XG : (g + 1) * XG, :, :],
                    )
                    chunks.extend(
                        (g * XG + tt, xTt, tt) for tt in range(0, XG, 2)
                    )
                return chunks

            DR = mybir.MatmulPerfMode.DoubleRow

            def mm_qkv(Qp, KVp, xTt, tt, t):
                # contraction pair (t, t+1): three DoubleRow terms
                x_hh = xTt[:, tt : tt + 2, 0, :]
                x_ll = xTt[:, tt : tt + 2, 1, :]
                w_hh = wTt[:, t : t + 2, 0, 0:EW]
                w_ll = wTt[:, t : t + 2, 1, 0:EW]
                v_hh = wTt[:, t : t + 2, 0, EW : EW + 2 * D]
                v_ll = wTt[:, t : t + 2, 1, EW : EW + 2 * D]
                first, last = t == 0, t == DD - 2
                nc.tensor.matmul(
                    Qp, x_hh, w_hh, start=first, stop=False, perf_mode=DR
                )
                nc.tensor.matmul(
                    Qp, x_ll, w_hh, start=False, stop=False, perf_mode=DR
                )
                nc.tensor.matmul(
                    Qp, x_hh, w_ll, start=False, stop=last, perf_mode=DR
                )
                nc.tensor.matmul(
                    KVp, x_hh, v_hh, start=first, stop=False, perf_mode=DR
                )
                nc.tensor.matmul(
                    KVp, x_ll, v_hh, start=False, stop=False, perf_mode=DR
                )
                nc.tensor.matmul(
                    KVp, x_hh, v_ll, start=False, stop=last, perf_mode=DR
                )

            for st in range(ST):
                cst = cspool.tile([P, EW], bf16, tag="cs")
                nc.sync.dma_start(out=cst, in_=cs[st * P : (st + 1) * P, :])

                if st == 0:
                    # s-tiles 0 and 1 interleave in half-contractions: while
                    # the second half of the weights streams in, the PE runs
                    # s-tile 1's first half on already-resident weights
                    chunks0 = [
                        (t, st0_x[t // XG0], t % XG0) for t in range(0, DD, 2)
                    ]
                    chunks1 = stream_x(1)
                    cst1 = cspool.tile([P, EW], bf16, tag="cs")
                    nc.sync.dma_start(out=cst1, in_=cs[P : 2 * P, :])
                    # second half of the weights streams behind s-tile 1's x,
                    # hidden under s-tile 1's first-half matmuls
                    for g in range(DD // 8, DD // 4):
                        nc.sync.dma_start(
                            out=wTt[:, 4 * g : 4 * g + 4, :, :],
                            in_=wTr[:, 4 * g : 4 * g + 4, :, :],
                        )
                    Qp0 = qps.tile([P, EW], f32, tag="Qp", name="Qp0")
                    KVp0 = kvps.tile([P, 2 * D], f32, tag="KVp", name="KVp0")
                    Qp1 = qps.tile([P, EW], f32, tag="Qp", name="Qp1")
                    KVp1 = kvps.tile([P, 2 * D], f32, tag="KVp", name="KVp1")
                    H = DD // 4  # half the pairs
                    for t, xTt, tt in chunks0[:H]:
                        mm_qkv(Qp0, KVp0, xTt, tt, t)
                    for t, xTt, tt in chunks1[:H]:
                        mm_qkv(Qp1, KVp1, xTt, tt, t)
                    for t, xTt, tt in chunks0[H:]:
                        mm_qkv(Qp0, KVp0, xTt, tt, t)
                    for t, xTt, tt in chunks1[H:]:
                        mm_qkv(Qp1, KVp1, xTt, tt, t)
                    later = [(0, Qp0, KVp0, cst), (1, Qp1, KVp1, cst1)]
                elif st == 1:
                    continue
                else:
                    Qp = qps.tile([P, EW], f32, tag="Qp")
                    KVp = kvps.tile([P, 2 * D], f32, tag="KVp")
                    for t, xTt, tt in stream_x(st):
                        mm_qkv(Qp, KVp, xTt, tt, t)
                    later = [(st, Qp, KVp, cst)]

                # rope via strided even/odd halves (2-level APs only - 3-level
                # APs overflow the fixed ISA instruction encoding).
                def ttr_ew(out, in0, in1, op):
                    nc.vector.tensor_tensor(out=out, in0=in0, in1=in1, op=op)

                A_ = mybir.AluOpType
                HF = EW // 2  # 256: cos table width for q
                for st_, Qp_, KVp_, cst_ in later:
                    rq = rpool.tile([P, EW], bf16, tag="rq")
                    t1 = rpool.tile([P, HF], f32, tag="t1")
                    t2 = rpool.tile([P, HF], f32, tag="t2")
                    cosr, sinr = cst_[:, 0:HF], cst_[:, HF : 2 * HF]

                    # K first: KVp frees early, so phase-2 psum tiles that
                    # land on kvps' recycled bytes don't wait on the last
                    # s-tile's q-rope
                    rk = rpool.tile([P, D], bf16, tag="rk")
                    k_ev, k_od = KVp_[:, 0:D:2], KVp_[:, 1:D:2]
                    cosk, sink = cst_[:, 0 : D // 2], cst_[:, HF : HF + D // 2]
                    ttr_ew(t1[:, 0 : D // 2], k_ev, cosk, A_.mult)
                    ttr_ew(t2[:, 0 : D // 2], k_od, sink, A_.mult)
                    ttr_ew(rk[:, 0:D:2], t1[:, 0 : D // 2], t2[:, 0 : D // 2], A_.subtract)
                    ttr_ew(t1[:, 0 : D // 2], k_ev, sink, A_.mult)
                    ttr_ew(t2[:, 0 : D // 2], k_od, cosk, A_.mult)
                    ttr_ew(rk[:, 1:D:2], t1[:, 0 : D // 2], t2[:, 0 : D // 2], A_.add)

                    # V -> bf16 [k, d] layout (ACT copy, cast, undo the x64
                    # weight scaling)
                    nc.scalar.activation(
                        out=Vt[:, st_, 0:D],
                        in_=KVp_[:, D : 2 * D],
                        func=mybir.ActivationFunctionType.Copy,
                        scale=inv64,
                    )
                    nc.sync.dma_start_transpose(
                        out=KTt[:, st_ * P : (st_ + 1) * P], in_=rk
                    )

                    q_ev, q_od = Qp_[:, 0:EW:2], Qp_[:, 1:EW:2]
                    ttr_ew(t1, q_ev, cosr, A_.mult)
                    ttr_ew(t2, q_od, sinr, A_.mult)
                    ttr_ew(rq[:, 0:EW:2], t1, t2, A_.subtract)
                    ttr_ew(t1, q_ev, sinr, A_.mult)
                    ttr_ew(t2, q_od, cosr, A_.mult)
                    ttr_ew(rq[:, 1:EW:2], t1, t2, A_.add)

                    # transpose rq (per head) into [d, s] via the DMA
                    # transpose engine (keeps PE free for matmuls)
                    nc.sync.dma_start_transpose(
                        out=QTt[:, :, st_ * P : (st_ + 1) * P], in_=rq
                    )

        # ---------------- phase 2: attention (transposed scores) --------------
        apool = ctx.enter_context(tc.tile_pool(name="apool", bufs=1))
        # split by head-pair so phase 3's first row-tile can start once
        # heads 0-1 finish, overlapping the rest of phase 2. A is stored as
        # fp8 hi/lo pairs for the compensated-fp8 output projection.
        Aall = [
            apool.tile([P, 2 * ST * D], f8, name=f"Aall{i}")
            for i in range(NH // 2)
        ]
        Aallr = [
            apool.tile([P, 2 * ST * D], f8, name=f"Aallr{i}")
            for i in range(NH // 2)
        ]
        # PSUM pool order matters: pools opened first reuse phase 1's freed
        # qps/kvps bytes and inherit a WAR on the last s-tile's rope reads.
        # ops (phase 3) and aps/dsps (needed a few steps into phase 2) absorb
        # that; sps (needed immediately) lands on fresh bytes.
        with (
            tc.tile_pool(name="ops", bufs=2, space="PSUM") as ops,
            tc.tile_pool(name="aps", bufs=2, space="PSUM") as aps,
            tc.tile_pool(name="sps", bufs=4, space="PSUM") as sps,
            tc.tile_pool(name="ptsb", bufs=5) as ptsb,
            tc.tile_pool(name="stat", bufs=8) as stat,
            tc.tile_pool(name="wopool", bufs=4) as wopool,
            tc.tile_pool(name="osb", bufs=2) as osb,
        ):
            steps = []
            for h in range(NH):
                for i in range(ST):
                    if plan[i]:
                        steps.append((h, i))

            # per-(h, qs) psum tiles holding 4 query-tiles' worth of slots;
            # accumulation groups are time-sequential so sharing one 2KB
            # zero-region is safe (earlier slots are only read afterwards)
            blk_tiles = {}

            def emit_front(step):
                """Scores (PE) + exp (ACT) + causal 0/1 multiply (DVE)."""
                h, i = step
                row = plan[i]
                PTt = ptsb.tile([P, ST, P], bf16, tag="PT")
                for c0 in range(0, len(row), 4):
                    chunk = row[c0 : c0 + 4]
                    S = sps.tile([P, 512], f32, tag="S")
                    for j, (kt, uid, uida) in enumerate(chunk):
                        nc.tensor.matmul(
                            S[:, j * P : (j + 1) * P],
                            KTt[:, kt * P : (kt + 1) * P],
                            QTt[:, h, i * P : (i + 1) * P],
                            start=True,
                            stop=True,
                        )
                        if uida >= 0:
                            nc.vector.tensor_add(
                                S[:, j * P : (j + 1) * P],
                                S[:, j * P : (j + 1) * P],
                                mbat[:, uida, :],
                            )
                    nc.scalar.activation(
                        out=PTt[:, c0 : c0 + len(chunk), :],
                        in_=S[:, 0 : len(chunk) * P],
                        func=mybir.ActivationFunctionType.Exp,
                        bias=ebias,
                    )
                    for j, (kt, uid, uida) in enumerate(chunk):
                        if uid >= 0:
                            nc.vector.tensor_tensor(
                                out=PTt[:, c0 + j, :],
                                in0=PTt[:, c0 + j, :],
                                in1=mbt[:, uid, :],
                                op=mybir.AluOpType.mult,
                            )
                return PTt

            def emit_back(step, PTt):
                """PV with fused denominator (PE) + recip (DVE) + normalized
                fp8 hi/lo Aall writes (DVE)."""
                h, i = step
                row = plan[i]
                qs, qi = i // 4, i % 4
                A = aps.tile([P, D + 4], f32, tag="A")
                nkt = len(row)
                for n, (kt, uid, uida) in enumerate(row):
                    nc.tensor.matmul(
                        A,
                        PTt[:, n, :],
                        Vt[:, kt, :],
                        start=(n == 0),
                        stop=(n == nkt - 1),
                    )
                rec = stat.tile([P, 1], f32, tag="rec")
                nc.vector.reciprocal(rec, A[:, D : D + 1])
                # Aall layout: [sp, (t*2 + dd)*128 + hb*64 + p] so the final
                # matmul's stationary slices are contiguous (walrus requires
                # a single free dim on weight APs)
                hb = h % 2

                def dv(Ah):
                    # dview[sp, p, dd] == Ah[:, i*256 + dd*128 + hb*64 + p]
                    return Ah[:, i * 2 * P : (i + 1) * 2 * P].rearrange(
                        "a (dd j) -> a dd j", dd=2
                    )[:, :, hb * 64 : hb * 64 + 64].rearrange(
                        "a dd p -> a p dd"
                    )

                dhi, dlo = dv(Aall[h // 2]), dv(Aallr[h // 2])
                Asl = A[:, 0:D].rearrange(
                    "a (p two) -> a p two", two=2
                )
                nc.vector.tensor_scalar_mul(dhi, Asl, rec)
                # lo = A*rec - hi (both fp8 rounded by the output dtype)
                nc.vector.scalar_tensor_tensor(
                    out=dlo,
                    in0=Asl,
                    scalar=rec,
                    in1=dhi,
                    op0=mybir.AluOpType.mult,
                    op1=mybir.AluOpType.subtract,
                )

            # zero Aall regions for fully-masked query rows (unreachable for
            # causal masks, but keeps the flatten well-defined). Emitted
            # before any phase-3 matmul can read them.
            for i in range(ST):
                if not plan[i]:
                    for h in range(NH):
                        for Ah in (Aall[h // 2], Aallr[h // 2]):
                            nc.vector.memset(
                                Ah[:, i * 2 * P : (i + 1) * 2 * P], 0.0
                            )

            # ---------------- phase 3 (interleaved into phase 2) -----------
            # Phase 2 is ACT(exp)-throughput-bound, leaving the PE with idle
            # slack between steps; phase-3 matmuls are drip-fed into that
            # slack as soon as their Aall inputs are final. wot loads are
            # emitted only when their pool buffer is provably free, so the
            # in-order SP queue never blocks on a WAR wait.
            wot_tiles = {}
            MC2 = DIM // 512
            W3 = 512

            def load_wot(mc):
                wot = wopool.tile(
                    [P, JT, 2, W3], f8, tag="wo", name=f"wot{mc}"
                )
                nc.sync.dma_start(out=wot, in_=woT[:, mc, :, :, :])
                wot_tiles[mc] = wot

            p3_queue = []  # (mc, it, u) units in emission order
            p3_open = {}
            pushed = set()
            # emitted at block close: which wot chunks to start loading
            # (only when their pool buffer is provably free)
            loads_at_close = {
                (0, 0): [3],
                (0, 1): [4],
                (1, 1): [5],
                (2, 1): [6],
                (3, 1): [7],
            }

            def close_p3_block(mc, it):
                O = p3_open.pop((mc, it))
                for k in loads_at_close.get((mc, it), []):
                    if k < MC2:
                        load_wot(k)
                if (mc, it) == (MC2 - 1, 1):
                    # final block: split the copy/store so the tail drains
                    # while the last half is still being copied
                    for half in range(2):
                        Ot = osb.tile([P, 256], bf16, tag="Oth")
                        nc.scalar.activation(
                            out=Ot,
                            in_=O[:, half * 256 : (half + 1) * 256],
                            func=mybir.ActivationFunctionType.Copy,
                            scale=inv64,
                        )
                        nc.sync.dma_start(
                            out=out[
                                it * P : (it + 1) * P,
                                mc * W3 + half * 256 : mc * W3 + (half + 1) * 256,
                            ],
                            in_=Ot,
                        )
                else:
                    Ot = osb.tile([P, W3], bf16, tag="Ot")
                    nc.scalar.activation(
                        out=Ot,
                        in_=O,
                        func=mybir.ActivationFunctionType.Copy,
                        scale=inv64,
                    )
                    nc.sync.dma_start(
                        out=out[it * P : (it + 1) * P, mc * W3 : (mc + 1) * W3],
                        in_=Ot,
                    )

            NU = 3 * JT // 2  # 48 DoubleRow units per block

            def pair_ap(Ah, ddj, t):
                idx = t * 2 + ddj
                return Ah.rearrange("a (tt j) -> a tt j", j=P)[
                    :, idx : idx + 3 : 2, :
                ]

            def push_block(mc, it):
                pushed.add((mc, it))
                p3_queue.extend([(mc, it, u) for u in range(NU)])

            def emit_p3(budget):
                emitted = 0
                while p3_queue and emitted < budget:
                    mc, it, u = p3_queue.pop(0)
                    key = (mc, it)
                    if key not in p3_open:
                        p3_open[key] = ops.tile(
                            [P, 512], f32, tag="O", name=f"O{mc}_{it}"
                        )
                    O = p3_open[key]
                    pi, term = u // 3, u % 3
                    t, ddj = 2 * (pi // 2), pi % 2
                    jt = ddj * ST + t
                    lhsT = pair_ap(
                        (Aall if term != 1 else Aallr)[it], ddj, t
                    )
                    rhs = wot_tiles[mc][:, jt : jt + 2, 1 if term == 2 else 0, :]
                    nc.tensor.matmul(
                        O,
                        lhsT,
                        rhs,
                        start=(u == 0),
                        stop=(u == NU - 1),
                        perf_mode=DR,
                    )
                    emitted += 1
                    if u == NU - 1:
                        close_p3_block(mc, it)
                return emitted

            # wot 0-2 transfer during heads 0-1, while the DMA device is idle
            load_wot(0)
            load_wot(1)
            load_wot(2)

            # Deep software pipeline: PE runs step n's scores while ACT/DVE
            # finish earlier steps, so the PE never waits on exp results
            DEPTH = 4
            pending = []

            all_rows = all(plan[i] for i in range(ST))

            def after_back(s0, front_step):
                h0_, i0_ = s0
                if h0_ == 1 and all_rows and i0_ % 2 == 1:
                    # block (0,0)'s jt-pair (t, t+1) is final once head 1 has
                    # written rows t and t+1; drip its 6 units in right here
                    pushed.add((0, 0))
                    pi0 = (i0_ // 2) * 2
                    for pi in (pi0, pi0 + 1):
                        p3_queue.extend(
                            [(0, 0, 3 * pi + tm) for tm in range(3)]
                        )
                # budget ~ the ACT-over-PE slack of the step the PE is
                # currently chewing on (one exp overhead per 4-kt chunk)
                nch = (len(plan[front_step[1]]) + 3) // 4 if front_step else 2
                emit_p3(max(3, min(7, 2 * nch + 1)))

            # blocks (1,0) and (2,0) become ready when heads 0-1 are done
            steps_h2 = [s for s in steps if s[0] == 2]
            steps_h3 = [s for s in steps if s[0] == 3]
            for step in steps:
                if steps_h2 and step == steps_h2[0]:
                    if not all_rows and (0, 0) not in pushed:
                        push_block(0, 0)
                    push_block(1, 0)
                if steps_h3 and step == steps_h3[0]:
                    push_block(2, 0)
                    push_block(3, 0)
                PTt = emit_front(step)
                pending.append((step, PTt))
                if len(pending) > DEPTH:
                    s0, p0 = pending.pop(0)
                    emit_back(s0, p0)
                    after_back(s0, step)
            for s0, p0 in pending:
                emit_back(s0, p0)
                after_back(s0, None)

            # remaining blocks; (0,1) first so wot buffer 0 frees early for
            # the just-in-time load of chunk 4
            base_rest = [(0, 1), (3, 0), (1, 1), (4, 0), (2, 1), (5, 0),
                         (3, 1), (6, 0), (4, 1), (7, 0), (5, 1), (6, 1),
                         (7, 1)]
            for mc, it in base_rest:
                if (mc, it) not in pushed:
                    push_block(mc, it)
            for mc in range(MC2):
                for it in range(ITILES):
                    if (mc, it) not in pushed:
                        push_block(mc, it)
            emit_p3(10 ** 9)

    nc.compile()
    return nc


def analyze_mask(mask, SEQ):
    """Classify 128x128 mask blocks: skip / free / masked.

    Masked blocks that only contain {0, -inf-ish} become 0/1 multiplicative
    blocks applied to exp'd scores (transposed, bf16). Blocks with other
    finite values become additive f32 blocks applied pre-exp (transposed).
    Returns (plan, mult_blocks, add_blocks); plan[i] is a list of
    (kt, uid_mult, uid_add).
    """
    ST = SEQ // P
    uniq_m, blocks_m = {}, []
    uniq_a, blocks_a = {}, []
    plan = []
    for i in range(ST):
        row = []
        for kt in range(ST):
            blk = mask[i * P : (i + 1) * P, kt * P : (kt + 1) * P]
            if (blk <= NEG_THRESH).all():
                continue
            if not blk.any():
                row.append((kt, -1, -1))
            elif ((blk == 0) | (blk <= NEG_THRESH)).all():
                key = blk.tobytes()
                if key not in uniq_m:
                    uniq_m[key] = len(blocks_m)
                    blocks_m.append(
                        np.ascontiguousarray((blk.T > NEG_THRESH).astype(np.float32))
                    )
                row.append((kt, uniq_m[key], -1))
            else:
                key = blk.tobytes()
                if key not in uniq_a:
                    uniq_a[key] = len(blocks_a)
                    blocks_a.append(np.ascontiguousarray(blk.T))
                row.append((kt, -1, uniq_a[key]))
        # fully masked query rows: leave empty; Aall is zero-filled for them
        plan.append(row)
    return plan, blocks_m, blocks_a


def make_rope_tables(cos_freq, sin_freq, SEQ, scale_quarter):
    """Build replicated [cos_rep (SEQ, NH*64) | sin_rep (SEQ, NH*64)] with
    sqrt(SCALE) folded in."""
    cos_t = np.tile(np.asarray(cos_freq, np.float32) * scale_quarter, (1, NH))
    sin_t = np.tile(np.asarray(sin_freq, np.float32) * scale_quarter, (1, NH))
    import ml_dtypes

    return np.ascontiguousarray(
        np.concatenate([cos_t, sin_t], axis=1).astype(ml_dtypes.bfloat16)
    )


_BUILD_CACHE = {}


def kernel(
    x,
    cos_freq,
    sin_freq,
    positions,
    mask,
    wq,
    wk,
    wv,
    wo,
    _trace=False,
):
    import sys

    if "/opt/trn_rl_repo" not in sys.path:
        sys.path.insert(0, "/opt/trn_rl_repo")
    from concourse.bass_utils import run_bass_kernel_spmd

    x = np.asarray(x, np.float32)
    mask = np.asarray(mask, np.float32)
    wq = np.asarray(wq, np.float32)
    wk = np.asarray(wk, np.float32)
    wv = np.asarray(wv, np.float32)
    wo = np.asarray(wo, np.float32)
    SEQ, DIM = x.shape
    assert wq.shape[0] == CORES * NH * D and wk.shape[0] == CORES * D
    assert 2 * SEQ == wq.shape[0], "flatten structure requires H*D == 2*SEQ"

    plan, blocks_m, blocks_a = analyze_mask(mask, SEQ)
    n_uniq, n_uniq_add = len(blocks_m), len(blocks_a)
    key = (SEQ, DIM, tuple(tuple(r) for r in plan))
    if key not in _BUILD_CACHE:
        _BUILD_CACHE[key] = build_attention_nc(SEQ, DIM, plan, n_uniq, n_uniq_add)
    nc = _BUILD_CACHE[key]

    import ml_dtypes

    bf16 = ml_dtypes.bfloat16
    f8 = ml_dtypes.float8_e4m3
    WSC = np.float32(64.0)  # weight pre-scale; undone via rope tables/V copy

    def f8hl(a):
        hi = a.astype(f8)
        lo = (a - hi.astype(np.float32)).astype(f8)
        return hi, lo

    # fold 1/64 into the rope tables (q and k both carry the x64 weights)
    scale_quarter = np.float32(D ** -0.25) / WSC
    cs = make_rope_tables(cos_freq, sin_freq, SEQ, scale_quarter)
    ST_, DD_ = SEQ // P, DIM // P
    xt = np.ascontiguousarray(x.reshape(ST_, P, DD_, P).transpose(3, 0, 2, 1))
    xh, xl = f8hl(xt)
    xT = np.ascontiguousarray(np.stack([xh, xl], axis=3))  # [p, st, t, 2, si]
    wot3 = np.ascontiguousarray(
        (WSC * wo.T).reshape(2 * SEQ // P, P, DIM).transpose(1, 0, 2)
    )  # [p, jt, m] = 64 * wo[m, jt*128+p]
    woh, wol = f8hl(wot3)
    JT_ = 2 * SEQ // P
    woT = np.ascontiguousarray(
        np.stack([woh, wol], axis=2)
        .reshape(P, JT_, 2, DIM // 512, 512)
        .transpose(0, 3, 1, 2, 4)
    )  # [p, mc, jt, hl, mi]
    if n_uniq:
        mbs = np.ascontiguousarray(np.stack(blocks_m, axis=0)).astype(bf16)
    else:
        mbs = np.zeros((1, P, P), bf16)
    if n_uniq_add:
        mbas = np.ascontiguousarray(np.stack(blocks_a, axis=0)).astype(np.float32)
    else:
        mbas = np.zeros((1, P, P), np.float32)

    in_maps = []
    for c in range(CORES):
        w_c = np.concatenate(
            [
                wq[c * NH * D : (c + 1) * NH * D],
                wk[c * D : (c + 1) * D],
                wv[c * D : (c + 1) * D],
            ],
            axis=0,
        )
        wt = np.ascontiguousarray(
            (WSC * w_c.T).reshape(DD_, P, -1).transpose(1, 0, 2)
        )  # [p, t, e] = 64 * w_c[e, t*128+p]
        wh, wl = f8hl(wt)
        whl = np.ascontiguousarray(np.stack([wh, wl], axis=2))
        in_maps.append(
            {
                "xT": xT,
                "wT": whl,
                "cs": cs,
                "maskb": mbs,
                "maskba": mbas,
                "woT": woT,
            }
        )

    import time as _time

    _t0 = _time.time()
    res = run_bass_kernel_spmd(nc, in_maps, list(range(CORES)), trace=_trace)
    global LAST_EXEC_NS
    LAST_EXEC_NS = int((_time.time() - _t0) * 1e9)
    outp = np.concatenate(
        [res.results[c]["out"] for c in range(CORES)], axis=0
    ).astype(np.float32)
    if _trace:
        return outp, res
    return outp



# revision 59
# speedup vs baseline: 1.0667x; 1.0667x over previous
"""Trainium2 Bass kernel for nn_Attention (GQA + RoPE + sliding-window mask).

Sharding: tensor-parallel over heads across 8 cores. Each core gets 4 q heads
and exactly 1 kv head (32 q / 8 kv heads, GQA group = 4). The reference's
quirky output flatten ((H,S,D)->(H,D,S)->reshape(S, H*D)) makes the final
projection contract over (d-parity, sequence) instead of heads, so the final
output is row-sharded by head block: core c produces rows [256c, 256c+256) of
the (2048, 4096) result with NO collective at all.

Per-core pipeline (all on one NeuronCore, same program on all 8 = pure SPMD):
  phase 1: QKV projections (bf16 matmuls) + RoPE (sqrt(scale) folded into the
           rope tables of both q and k) + DMA transposes into [d, s] layouts.
  phase 2: TRANSPOSED attention. Scores are computed as S^T[k, q] directly
           (K^T tile stationary, Q^T moving), so the exp'd probabilities land
           in SBUF already in the [k, q] layout PV needs - no P transposes.
           Softmax uses no running max (logits are O(10), exp biased by -8
           stays in range); denominators are per-q partition sums computed
           with free 1-wide ones-matmuls on the PE; causal masking is a 0/1
           triangular multiply on the bf16 P tile (DVE). PV then produces
           A[q, d] directly, normalized into the Aall layout by ACT.
  phase 3: final projection vs full wo (bf16), row slice out.
"""

import numpy as np
from contextlib import ExitStack

P = 128
D = 128  # head dim
NH = 4   # q heads per core
CORES = 8
NEG_THRESH = -1e8
EXP_BIAS = -8.0  # constant bias inside exp; cancels in normalization


def build_attention_nc(
    SEQ,
    DIM,
    plan,
    n_uniq,
    n_uniq_add=0,
):
    """Build the per-core Bass program.

    plan: list over q-tiles i (SEQ//128 entries) of lists of (kt, uid, uid_add)
          at 128x128 block granularity. uid == -1: no masking needed.
          uid >= 0: multiply the exp'd P tile by 0/1 block `uid` (DVE).
          uid_add >= 0: add f32 block `uid_add` to scores before exp (general
          additive masks; unused for causal). Blocks absent are fully masked.
    """
    import concourse.bass as bass
    import concourse.bacc as bacc
    import concourse.mybir as mybir
    import concourse.tile as tile

    f32 = mybir.dt.float32
    bf16 = mybir.dt.bfloat16

    ST = SEQ // P          # 16 s-tiles
    DD = DIM // P          # 32 contraction tiles
    EW = NH * D            # 512 q-projection width
    JT = 2 * SEQ // P      # 32 j-tiles for final matmul
    MC = DIM // 512        # 8 output chunks
    ITILES = (NH * 64) // P  # 2 output row tiles
    assert NH == 4 and SEQ % 512 == 0 and DIM % 512 == 0

    nc = bacc.Bacc(trn_type="TRN2", debug=False, num_devices=CORES)

    f8 = mybir.dt.float8e4

    # x and the QKV weights arrive as packed fp8 hi/lo pairs (hi = fp8(v),
    # lo = fp8(v - hi)); three DoubleRow matmuls per contraction-tile pair
    # compute hi*hi + lo*hi + hi*lo at 0.75x the bf16 cycle cost with ~2x
    # BETTER accuracy. Weights are host-scaled by 64 so the lo residuals
    # stay above fp8's subnormal floor; the 1/64 is folded into the rope
    # tables and the V copy.
    # xT[p, st, t, hl, si] = fp8hl(x[st*128+si, t*128+p])
    xT = nc.dram_tensor(
        "xT", [P, ST, DD, 2, P], f8, kind="ExternalInput"
    ).ap()
    # wT[p, t, hl, e] = fp8hl(64 * w_c[e, t*128+p])
    wT = nc.dram_tensor(
        "wT", [P, DD, 2, EW + 2 * D], f8, kind="ExternalInput"
    ).ap()
    cs = nc.dram_tensor("cs", [SEQ, EW], bf16, kind="ExternalInput").ap()
    mb = nc.dram_tensor(
        "maskb", [max(n_uniq, 1), P, P], bf16, kind="ExternalInput"
    ).ap()
    mba = nc.dram_tensor(
        "maskba", [max(n_uniq_add, 1), P, P], f32, kind="ExternalInput"
    ).ap()
    # woT[p, mc, jt, hl, mi] = fp8hl(64 * wo[mc*256+mi, jt*128+p]) -
    # chunk-major so each 256-wide chunk load is one contiguous run per
    # partition (full DMA rate)
    woT = nc.dram_tensor(
        "woT", [P, DIM // 512, JT, 2, 512], f8, kind="ExternalInput"
    ).ap()
    out = nc.dram_tensor("out", [NH * 64, DIM], bf16, kind="ExternalOutput").ap()

    with tile.TileContext(nc) as tc, ExitStack() as ctx:
        const = ctx.enter_context(tc.tile_pool(name="const", bufs=1))
        ones = const.tile([P, 1], bf16)
        nc.vector.memset(ones, 1.0)
        ebias = const.tile([P, 1], f32)
        nc.vector.memset(ebias, EXP_BIAS)
        # touch Exp at t=0 so the ACT table load doesn't stall phase 2
        scr = const.tile([P, 1], f32)
        nc.scalar.activation(
            out=scr, in_=ebias, func=mybir.ActivationFunctionType.Exp
        )
        inv64 = const.tile([P, 1], f32)
        nc.vector.memset(inv64, 1.0 / 64.0)

        pers = ctx.enter_context(tc.tile_pool(name="pers", bufs=1))
        QTt = pers.tile([P, NH, ST * P], bf16)   # [d, h, s]
        KTt = pers.tile([P, ST * P], bf16)       # [d, s]
        Vt = pers.tile([P, ST, D + 4], bf16)     # [k(part), ktile, d | 1 pad]

        # wo streams in 8KB quarter-chunks ([mc, ddj-half, t-quarter]) into a
        # ring that lives for the whole kernel, so ~12 quarters prefetch into
        # phase 1's spare DMA bandwidth and phase 3 never stalls on wo loads.
        W3 = 512
        MC2 = DIM // 512
        wopool = ctx.enter_context(tc.tile_pool(name="wopool", bufs=13))
        wot_tiles = {}
        # consumption order within an mc: (ddj0,q0), (ddj1,q0), (ddj0,q1), (ddj1,q1)
        QUARTER_ORDER = ((0, 0), (1, 0), (0, 1), (1, 1))
        # all 32 quarters in consumption order; loads pop from the front so
        # arrival order always matches need order
        wo_queue = [
            (mc, dd, qq) for mc in range(MC2) for dd, qq in QUARTER_ORDER
        ]

        def load_next_wo(pool, n):
            for _ in range(n):
                if not wo_queue:
                    return
                mc, dd, qq = wo_queue.pop(0)
                wot = pool.tile(
                    [P, 8, 2, W3], f8, tag="wo", name=f"wot{mc}_{dd}_{qq}"
                )
                nc.sync.dma_start(
                    out=wot,
                    in_=woT[:, mc, dd * ST + qq * 8 : dd * ST + (qq + 1) * 8, :, :],
                )
                wot_tiles[(mc, dd, qq)] = wot

        # phase-1 prefetch: one quarter per phase-1b s-tile (mc 0-2
        # resident by phase-1 end) — one 3.2us transfer fits each s-tile's
        # spare DMA bandwidth without starving the next s-tile's x
        WO_PREFETCH_ST = tuple(range(4, 16))
        # col D is all-ones: PV's rhs [V | 1] also accumulates the softmax
        # denominator into the A psum's column D
        nc.vector.memset(Vt[:, :, D : D + 1], 1.0)
        nc.vector.memset(Vt[:, :, D + 1 : D + 4], 0.0)
        if n_uniq > 0:
            mbt = pers.tile([P, n_uniq, P], bf16)
        if n_uniq_add > 0:
            mbat = pers.tile([P, n_uniq_add, P], f32)

        # ---------------- phase 1: projections + rope + layout ----------------
        with (
            tc.tile_pool(name="wpool", bufs=1) as wpool,
            tc.tile_pool(name="xpool", bufs=4) as xpool,
            tc.tile_pool(name="cspool", bufs=4) as cspool,
            tc.tile_pool(name="rpool", bufs=2) as rpool,
            tc.tile_pool(name="qps", bufs=4, space="PSUM") as qps,
            tc.tile_pool(name="kvps", bufs=4, space="PSUM") as kvps,
        ):
            wTt = wpool.tile([P, DD, 2, EW + 2 * D], f8)
            wTr = wT

            XG = min(8, DD)  # dd-tiles per streamed x chunk
            xTr = xT
            # Phase 1a streams the weights ONCE while consuming them
            # pair-major across the first NA s-tiles: the startup window is
            # pure DMA capacity (w 6.3MB + x), and s-tile-major order would
            # leave the PE idle ~half of it waiting for late weight pairs.
            NA = 4
            # Per-round x chunks for the NA s-tiles with the round's weight
            # pieces riding along. DMA emission is DEFERRED and woven into
            # the matmul emission below: the xT0 ring has 6 slots for 16
            # chunks, so a chunk's dma_start must be emitted only after the
            # matmuls reading its ring-predecessor exist, or the WAR is lost
            # and the transfer overwrites live data.
            xa = [[None] * (DD // 2) for _ in range(NA)]

            def emit_xa_dma(c, s):
                xTt = xpool.tile([P, XG, 2, P], f8, tag="xT0", bufs=6)
                nc.sync.dma_start(
                    out=xTt, in_=xTr[:, s, c * XG : (c + 1) * XG, :, :]
                )
                for tt in range(0, XG, 2):
                    xa[s][(c * XG + tt) // 2] = (c * XG + tt, xTt, tt)
                if c == 0:
                    # round 0: one weight pair after each x chunk, so the
                    # first pairs' operands land just-in-time
                    w0 = 2 * s
                    nc.sync.dma_start(
                        out=wTt[:, w0 : w0 + 2, :, :],
                        in_=wTr[:, w0 : w0 + 2, :, :],
                    )
                elif s in (0, 2):
                    w0 = c * XG + (s // 2) * (XG // 2)
                    nc.sync.dma_start(
                        out=wTt[:, w0 : w0 + XG // 2, :, :],
                        in_=wTr[:, w0 : w0 + XG // 2, :, :],
                    )

            def stream_x(st):
                chunks = []
                for g in range(DD // XG):
                    xTt = xpool.tile([P, XG, 2, P], f8, tag="xT")
                    nc.sync.dma_start(
                        out=xTt,
                        in_=xTr[:, st, g * XG : (g + 1) * XG, :, :],
                    )
                    chunks.extend(
                        (g * XG + tt, xTt, tt) for tt in range(0, XG, 2)
                    )
                return chunks

            DR = mybir.MatmulPerfMode.DoubleRow

            def mm_qkv(Qp, KVp, xTt, tt, t):
                # contraction pair (t, t+1): three DoubleRow terms. The two
                # hi-weight terms of BOTH psums come before the lo-weight
                # terms, so on the in-order PE queue the w-hi half of a
                # streamed weight pair enables 4 of 6 matmuls immediately.
                x_hh = xTt[:, tt : tt + 2, 0, :]
                x_ll = xTt[:, tt : tt + 2, 1, :]
                w_hh = wTt[:, t : t + 2, 0, 0:EW]
                w_ll = wTt[:, t : t + 2, 1, 0:EW]
                v_hh = wTt[:, t : t + 2, 0, EW : EW + 2 * D]
                v_ll = wTt[:, t : t + 2, 1, EW : EW + 2 * D]
                first, last = t == 0, t == DD - 2
                nc.tensor.matmul(
                    Qp, x_hh, w_hh, start=first, stop=False, perf_mode=DR
                )
                nc.tensor.matmul(
                    Qp, x_ll, w_hh, start=False, stop=False, perf_mode=DR
                )
                nc.tensor.matmul(
                    KVp, x_hh, v_hh, start=first, stop=False, perf_mode=DR
                )
                nc.tensor.matmul(
                    KVp, x_ll, v_hh, start=False, stop=False, perf_mode=DR
                )
                nc.tensor.matmul(
                    Qp, x_hh, w_ll, start=False, stop=last, perf_mode=DR
                )
                nc.tensor.matmul(
                    KVp, x_hh, v_ll, start=False, stop=last, perf_mode=DR
                )

            def mm_kv_only(KVp, xTt, tt, t):
                x_hh = xTt[:, tt : tt + 2, 0, :]
                x_ll = xTt[:, tt : tt + 2, 1, :]
                v_hh = wTt[:, t : t + 2, 0, EW : EW + 2 * D]
                v_ll = wTt[:, t : t + 2, 1, EW : EW + 2 * D]
                first, last = t == 0, t == DD - 2
                nc.tensor.matmul(
                    KVp, x_hh, v_hh, start=first, stop=False, perf_mode=DR
                )
                nc.tensor.matmul(
                    KVp, x_ll, v_hh, start=False, stop=False, perf_mode=DR
                )
                nc.tensor.matmul(
                    KVp, x_hh, v_ll, start=False, stop=last, perf_mode=DR
                )

            def mm_q_only(Qp, xTt, tt, t):
                x_hh = xTt[:, tt : tt + 2, 0, :]
                x_ll = xTt[:, tt : tt + 2, 1, :]
                w_hh = wTt[:, t : t + 2, 0, 0:EW]
                w_ll = wTt[:, t : t + 2, 1, 0:EW]
                first, last = t == 0, t == DD - 2
                nc.tensor.matmul(
                    Qp, x_hh, w_hh, start=first, stop=False, perf_mode=DR
                )
                nc.tensor.matmul(
                    Qp, x_ll, w_hh, start=False, stop=False, perf_mode=DR
                )
                nc.tensor.matmul(
                    Qp, x_hh, w_ll, start=False, stop=last, perf_mode=DR
                )

            # rope via strided even/odd halves (2-level APs only - 3-level
            # APs overflow the fixed ISA instruction encoding).
            def ttr_ew(out, in0, in1, op):
                nc.vector.tensor_tensor(out=out, in0=in0, in1=in1, op=op)

            A_ = mybir.AluOpType
            HF = EW // 2  # 256: cos table width for q

            def rope_k(st_, KVp_, cst_):
                t1 = rpool.tile([P, D // 2], f32, tag="t1")
                t2 = rpool.tile([P, D // 2], f32, tag="t2")
                rk = rpool.tile([P, D], bf16, tag="rk")
                k_ev, k_od = KVp_[:, 0:D:2], KVp_[:, 1:D:2]
                cosk, sink = cst_[:, 0 : D // 2], cst_[:, HF : HF + D // 2]
                ttr_ew(t1, k_ev, cosk, A_.mult)
                ttr_ew(t2, k_od, sink, A_.mult)
                ttr_ew(rk[:, 0:D:2], t1, t2, A_.subtract)
                ttr_ew(t1, k_ev, sink, A_.mult)
                ttr_ew(t2, k_od, cosk, A_.mult)
                ttr_ew(rk[:, 1:D:2], t1, t2, A_.add)

                # V -> bf16 [k, d] layout (ACT copy, cast, undo the x64
                # weight scaling)
                nc.scalar.activation(
                    out=Vt[:, st_, 0:D],
                    in_=KVp_[:, D : 2 * D],
                    func=mybir.ActivationFunctionType.Copy,
                    scale=inv64,
                )
                nc.sync.dma_start_transpose(
                    out=KTt[:, st_ * P : (st_ + 1) * P], in_=rk
                )

            def rope_q(st_, Qp_, cst_):
                rq = rpool.tile([P, EW], bf16, tag="rq")
                t1 = rpool.tile([P, HF], f32, tag="t1")
                t2 = rpool.tile([P, HF], f32, tag="t2")
                cosr, sinr = cst_[:, 0:HF], cst_[:, HF : 2 * HF]
                q_ev, q_od = Qp_[:, 0:EW:2], Qp_[:, 1:EW:2]
                ttr_ew(t1, q_ev, cosr, A_.mult)
                ttr_ew(t2, q_od, sinr, A_.mult)
                ttr_ew(rq[:, 0:EW:2], t1, t2, A_.subtract)
                ttr_ew(t1, q_ev, sinr, A_.mult)
                ttr_ew(t2, q_od, cosr, A_.mult)
                ttr_ew(rq[:, 1:EW:2], t1, t2, A_.add)

                # transpose rq (per head) into [d, s] via the DMA
                # transpose engine (keeps PE free for matmuls)
                nc.sync.dma_start_transpose(
                    out=QTt[:, :, st_ * P : (st_ + 1) * P], in_=rq
                )

            # ---- phase 1a: s-tiles 0..NA-1 pair-major vs the streaming w:
            # each weight pair is consumed against NA s-tiles as it lands,
            # so the PE tracks the DMA-capacity-bound startup window instead
            # of idling for late pairs
            Qpa = [
                qps.tile([P, EW], f32, tag="Qp", name=f"Qpa{s}")
                for s in range(NA)
            ]
            KVpa = [
                kvps.tile([P, 2 * D], f32, tag="KVp", name=f"KVpa{s}")
                for s in range(NA)
            ]
            # fresh xT0 slots: round 0 + half of round 1
            for s in range(NA):
                emit_xa_dma(0, s)
            emit_xa_dma(1, 0)
            emit_xa_dma(1, 1)
            # masks are tiny; land them long before phase 2 needs them
            if n_uniq > 0:
                nc.sync.dma_start(out=mbt, in_=mb.rearrange("u p m -> p u m"))
            if n_uniq_add > 0:
                nc.sync.dma_start(out=mbat, in_=mba.rearrange("u p m -> p u m"))
            pending_dma = [(1, 2), (1, 3)] + [
                (c, s) for c in (2, 3) for s in range(NA)
            ]
            for rnd in range(DD // XG):
                for pr in range(rnd * 4, rnd * 4 + 4):
                    for s in range(NA):
                        t, xTt, tt = xa[s][pr]
                        mm_qkv(Qpa[s], KVpa[s], xTt, tt, t)
                # ring slots of the next chunks now have their readers
                # emitted; release the next DMAs
                for _ in range(4 if rnd < 2 else 2):
                    if pending_dma:
                        emit_xa_dma(*pending_dma.pop(0))
            cs_a = []
            for s in range(NA):
                csa = cspool.tile([P, EW], bf16, tag="cs")
                nc.sync.dma_start(out=csa, in_=cs[s * P : (s + 1) * P, :])
                cs_a.append(csa)
            for s in range(NA):
                rope_k(s, KVpa[s], cs_a[s])
                rope_q(s, Qpa[s], cs_a[s])

            # ---- phase 1b: remaining s-tiles, s-tile-major ----
            for st in range(NA, ST):
                cst = cspool.tile([P, EW], bf16, tag="cs")
                nc.sync.dma_start(out=cst, in_=cs[st * P : (st + 1) * P, :])

                if st >= ST - 3:
                    # last s-tiles: run the whole KV contraction first and
                    # rope/transpose K before the Q matmuls, so the K tiles
                    # phase 2's first score chunks need are ready ~5us
                    # earlier (the K transpose rides the in-order DMA queue)
                    Qp = qps.tile([P, EW], f32, tag="Qp")
                    KVp = kvps.tile([P, 2 * D], f32, tag="KVp")
                    chunks = stream_x(st)
                    for t, xTt, tt in chunks:
                        mm_kv_only(KVp, xTt, tt, t)
                    rope_k(st, KVp, cst)
                    for t, xTt, tt in chunks:
                        mm_q_only(Qp, xTt, tt, t)
                    later = [(st, Qp, None, cst)]
                else:
                    Qp = qps.tile([P, EW], f32, tag="Qp")
                    KVp = kvps.tile([P, 2 * D], f32, tag="KVp")
                    for t, xTt, tt in stream_x(st):
                        mm_qkv(Qp, KVp, xTt, tt, t)
                    later = [(st, Qp, KVp, cst)]

                for st_, Qp_, KVp_, cst_ in later:
                    if KVp_ is not None:
                        rope_k(st_, KVp_, cst_)
                    rope_q(st_, Qp_, cst_)

                if st in WO_PREFETCH_ST:
                    load_next_wo(wopool, 1)

        # ---------------- phase 2: attention (transposed scores) --------------
        apool = ctx.enter_context(tc.tile_pool(name="apool", bufs=1))
        # split by head-pair so phase 3's first row-tile can start once
        # heads 0-1 finish, overlapping the rest of phase 2. A is stored as
        # fp8 hi/lo pairs for the compensated-fp8 output projection.
        Aall = [
            apool.tile([P, 2 * ST * D], f8, name=f"Aall{i}")
            for i in range(NH // 2)
        ]
        Aallr = [
            apool.tile([P, 2 * ST * D], f8, name=f"Aallr{i}")
            for i in range(NH // 2)
        ]
        # PSUM pool order matters: pools opened first reuse phase 1's freed
        # qps/kvps bytes and inherit a WAR on the last s-tile's rope reads.
        # ops (phase 3) and aps/dsps (needed a few steps into phase 2) absorb
        # that; sps (needed immediately) lands on fresh bytes.
        with (
            tc.tile_pool(name="ops", bufs=2, space="PSUM") as ops,
            tc.tile_pool(name="aps", bufs=2, space="PSUM") as aps,
            tc.tile_pool(name="sps", bufs=2, space="PSUM") as sps,
            tc.tile_pool(name="ptsb", bufs=4) as ptsb,
            tc.tile_pool(name="stat", bufs=8) as stat,
            tc.tile_pool(name="osb", bufs=4) as osb,
            tc.tile_pool(name="wopoolB", bufs=5) as wopoolB,
        ):
            # q-tiles processed long-rows-first so the front/back pipeline
            # fills with real work instead of stalling on semaphore chains
            # through tiny rows at phase-2 entry. Head 0 starts at i=12 (not
            # 15) because q-tile 15's transpose is still draining through the
            # DMA queue when phase 2 begins.
            steps = []
            h0_order = [10, 11, 12, 13, 14, 15] + list(range(9, -1, -1))
            for h in range(NH):
                for i in (h0_order if h == 0 else reversed(range(ST))):
                    if plan[i]:
                        steps.append((h, i))

            # per-(h, qs) psum tiles holding 4 query-tiles' worth of slots;
            # accumulation groups are time-sequential so sharing one 2KB
            # zero-region is safe (earlier slots are only read afterwards)
            blk_tiles = {}

            def emit_front(step):
                """Scores (PE) + exp (ACT) + causal 0/1 multiply (DVE).

                8-block exp chunks (2-bank PSUM S tiles): ACT instruction
                count halves, and ACT's ~185ns per-instruction access
                latency is what makes phase 2 ACT-critical."""
                h, i = step
                row = plan[i]
                PTt = ptsb.tile([P, ST, P], bf16, tag="PT")
                for c0 in range(0, len(row), 8):
                    chunk = row[c0 : c0 + 8]
                    S = sps.tile([P, 1024], f32, tag="S")
                    for j, (kt, uid, uida) in enumerate(chunk):
                        nc.tensor.matmul(
                            S[:, j * P : (j + 1) * P],
                            KTt[:, kt * P : (kt + 1) * P],
                            QTt[:, h, i * P : (i + 1) * P],
                            start=True,
                            stop=True,
                        )
                        if uida >= 0:
                            nc.vector.tensor_add(
                                S[:, j * P : (j + 1) * P],
                                S[:, j * P : (j + 1) * P],
                                mbat[:, uida, :],
                            )
                    nc.scalar.activation(
                        out=PTt[:, c0 : c0 + len(chunk), :],
                        in_=S[:, 0 : len(chunk) * P],
                        func=mybir.ActivationFunctionType.Exp,
                        bias=ebias,
                    )
                    for j, (kt, uid, uida) in enumerate(chunk):
                        if uid >= 0:
                            nc.vector.tensor_tensor(
                                out=PTt[:, c0 + j, :],
                                in0=PTt[:, c0 + j, :],
                                in1=mbt[:, uid, :],
                                op=mybir.AluOpType.mult,
                            )
                return PTt

            def emit_back(step, PTt):
                """PV with fused denominator (PE) + recip (DVE) + normalized
                fp8 hi/lo Aall writes (DVE)."""
                h, i = step
                row = plan[i]
                qs, qi = i // 4, i % 4
                A = aps.tile([P, D + 4], f32, tag="A")
                nkt = len(row)
                for n, (kt, uid, uida) in enumerate(row):
                    nc.tensor.matmul(
                        A,
                        PTt[:, n, :],
                        Vt[:, kt, :],
                        start=(n == 0),
                        stop=(n == nkt - 1),
                    )
                rec = stat.tile([P, 1], f32, tag="rec")
                nc.vector.reciprocal(rec, A[:, D : D + 1])
                # Aall layout: [sp, (t*2 + dd)*128 + hb*64 + p] so the final
                # matmul's stationary slices are contiguous (walrus requires
                # a single free dim on weight APs)
                hb = h % 2

                def dv(Ah):
                    # dview[sp, p, dd] == Ah[:, i*256 + dd*128 + hb*64 + p]
                    return Ah[:, i * 2 * P : (i + 1) * 2 * P].rearrange(
                        "a (dd j) -> a dd j", dd=2
                    )[:, :, hb * 64 : hb * 64 + 64].rearrange(
                        "a dd p -> a p dd"
                    )

                dhi, dlo = dv(Aall[h // 2]), dv(Aallr[h // 2])
                Asl = A[:, 0:D].rearrange(
                    "a (p two) -> a p two", two=2
                )
                nc.vector.tensor_scalar_mul(dhi, Asl, rec)
                # lo = A*rec - hi (both fp8 rounded by the output dtype)
                nc.vector.scalar_tensor_tensor(
                    out=dlo,
                    in0=Asl,
                    scalar=rec,
                    in1=dhi,
                    op0=mybir.AluOpType.mult,
                    op1=mybir.AluOpType.subtract,
                )

            # zero Aall regions for fully-masked query rows (unreachable for
            # causal masks, but keeps the flatten well-defined). Emitted
            # before any phase-3 matmul can read them.
            for i in range(ST):
                if not plan[i]:
                    for h in range(NH):
                        for Ah in (Aall[h // 2], Aallr[h // 2]):
                            nc.vector.memset(
                                Ah[:, i * 2 * P : (i + 1) * 2 * P], 0.0
                            )

            # ---------------- phase 3 (interleaved into phase 2) -----------
            # Phase 2 is ACT(exp)-throughput-bound, leaving the PE with idle
            # slack between steps; phase-3 matmuls are drip-fed into that
            # slack as soon as their Aall inputs are final. wo quarters for
            # mc 0-2 prefetched during phase 1; the rest ride a close-
            # triggered ring (close of (mc, 1) frees mc's 4 quarters → load
            # mc+3's), so the in-order SP queue never blocks on a WAR wait.
            p3_queue = []  # (mc, it, u) units in emission order
            p3_open = {}
            p3_cnt = {}  # units emitted per psum key; drives start/stop
            pushed = set()

            # the very last block runs as two 256-wide half-accumulations so
            # the first half's copy/store overlaps the second half's matmuls
            FINAL = (MC2 - 1, 1)

            def close_p3_block(mc, it, half):
                O = p3_open.pop((mc, it, half))
                if (mc, it) == (3, 1):
                    # (3,1) closing frees wopoolB's mc3 slots + wopool's
                    # spare; later closes free wopool's mc0-2 slots in ring
                    # order, so pops always land where the WAR clears next
                    load_next_wo(wopoolB, 3)
                    load_next_wo(wopool, 1)
                elif it == 1 and half != 0:
                    load_next_wo(wopool, 4)
                base = mc * W3 + (half * 256 if half is not None else 0)
                width = 256 if half is not None else W3
                npc = 2 if half is not None else 1
                w = width // npc
                for pc in range(npc):
                    Ot = osb.tile([P, w], bf16, tag="Ot")
                    nc.scalar.activation(
                        out=Ot,
                        in_=O[:, pc * w : (pc + 1) * w],
                        func=mybir.ActivationFunctionType.Copy,
                        scale=inv64,
                    )
                    nc.sync.dma_start(
                        out=out[
                            it * P : (it + 1) * P,
                            base + pc * w : base + (pc + 1) * w,
                        ],
                        in_=Ot,
                    )

            NU = 3 * JT // 2  # 48 DoubleRow units per block

            def pair_ap(Ah, ddj, t):
                idx = t * 2 + ddj
                return Ah.rearrange("a (tt j) -> a tt j", j=P)[
                    :, idx : idx + 3 : 2, :
                ]

            def push_block(mc, it):
                pushed.add((mc, it))
                n = 2 * NU if (mc, it) == FINAL else NU
                p3_queue.extend([(mc, it, u) for u in range(n)])

            def emit_p3(budget):
                emitted = 0
                while p3_queue and emitted < budget:
                    mc, it, u = p3_queue.pop(0)
                    if (mc, it) == FINAL:
                        half, uu = divmod(u, NU)
                    else:
                        half, uu = None, u
                    key = (mc, it, half)
                    if key not in p3_open:
                        p3_open[key] = ops.tile(
                            [P, 256 if half is not None else 512],
                            f32,
                            tag="O",
                            name=f"O{mc}_{it}_{half}",
                        )
                    O = p3_open[key]
                    pi, term = uu // 3, uu % 3
                    t, ddj = 2 * (pi // 2), pi % 2
                    lhsT = pair_ap(
                        (Aall if term != 1 else Aallr)[it], ddj, t
                    )
                    rhs = wot_tiles[(mc, ddj, t // 8)][
                        :, t % 8 : t % 8 + 2, 1 if term == 2 else 0, :
                    ]
                    if half is not None:
                        rhs = rhs[:, :, half * 256 : (half + 1) * 256]
                    # drip-fed units arrive out of unit-id order, so the
                    # psum group's start (zeroing) / stop must track the
                    # EMISSION count, not the unit id
                    cnt = p3_cnt.get(key, 0)
                    p3_cnt[key] = cnt + 1
                    nc.tensor.matmul(
                        O,
                        lhsT,
                        rhs,
                        start=(cnt == 0),
                        stop=(cnt == NU - 1),
                        perf_mode=DR,
                    )
                    emitted += 1
                    if cnt == NU - 1:
                        close_p3_block(mc, it, half)
                return emitted

            # stage the next 5 quarters (3_0_0..4_0_0) into the spare wopool
            # slot + the fresh wopoolB ring right at phase-2 start: fresh
            # slots have no WAR, so these transfers run in phase 2's
            # otherwise-idle DMA window
            load_next_wo(wopool, 1)
            load_next_wo(wopoolB, 5)

            # Deep software pipeline: PE runs step n's scores while ACT/DVE
            # finish earlier steps, so the PE never waits on exp results
            DEPTH = 3
            pending = []

            all_rows = all(plan[i] for i in range(ST))

            steps_h1 = [s for s in steps if s[0] == 1]

            def after_back(s0, front_step):
                h0_, i0_ = s0
                if steps_h1 and s0 == steps_h1[-1]:
                    # ALL of head 0-1's PV writes are now emitted: (1,0)'s
                    # units may be queued without reading not-yet-written
                    # Aall[0] rows (pushing at head-2's first FRONT would
                    # race the last DEPTH backs of head 1)
                    if not all_rows and (0, 0) not in pushed:
                        push_block(0, 0)
                    push_block(1, 0)
                if h0_ == 1 and all_rows and i0_ % 2 == 0:
                    # with descending q-tile order, rows (i0, i0+1) are both
                    # final once head 1 reaches even i0; drip block (0,0)'s
                    # matching jt-pair units in right here
                    pushed.add((0, 0))
                    for pi in (i0_, i0_ + 1):
                        p3_queue.extend(
                            [(0, 0, 3 * pi + tm) for tm in range(3)]
                        )
                if h0_ == 3 and all_rows and i0_ % 2 == 0:
                    # same for (0,1) as head 3 completes its rows: closing
                    # (0,1) at the very start of the tail frees mc0's wopool
                    # slots, so the mc4-7 load chain starts ~5us earlier
                    pushed.add((0, 1))
                    for pi in (i0_, i0_ + 1):
                        p3_queue.extend(
                            [(0, 1, 3 * pi + tm) for tm in range(3)]
                        )
                # budget ~ the ACT-over-PE slack of the step the PE is
                # currently chewing on
                nch = (len(plan[front_step[1]]) + 3) // 4 if front_step else 2
                emit_p3(max(3, min(9, 2 * nch + 3)))

            # blocks (1,0) and (2,0) become ready when heads 0-1 are done
            # (mc 0-2 wo quarters are phase-1-prefetched; mc 3+ stay in the
            # tail where the close-triggered ring covers them)
            steps_h2 = [s for s in steps if s[0] == 2]
            steps_h3 = [s for s in steps if s[0] == 3]
            for step in steps:
                if steps_h3 and step == steps_h3[0]:
                    push_block(2, 0)
                    push_block(3, 0)
                PTt = emit_front(step)
                pending.append((step, PTt))
                if len(pending) > DEPTH:
                    s0, p0 = pending.pop(0)
                    emit_back(s0, p0)
                    after_back(s0, step)
            for s0, p0 in pending:
                emit_back(s0, p0)
                after_back(s0, None)

            # remaining blocks: it=1 of mc0-3 first — their closes free the
            # wopool/wopoolB slots for mc4-7's ring loads in consumption
            # order ((0,1) leftovers drain first from the head-3 drip)
            base_rest = [(0, 1), (1, 1), (2, 1), (3, 1), (4, 0), (4, 1),
                         (5, 0), (5, 1), (6, 0), (6, 1), (7, 0), (7, 1),
                         (3, 0)]
            for mc, it in base_rest:
                if (mc, it) not in pushed:
                    push_block(mc, it)
            for mc in range(MC2):
                for it in range(ITILES):
                    if (mc, it) not in pushed:
                        push_block(mc, it)
            emit_p3(10 ** 9)

    nc.compile()
    return nc


def analyze_mask(mask, SEQ):
    """Classify 128x128 mask blocks: skip / free / masked.

    Masked blocks that only contain {0, -inf-ish} become 0/1 multiplicative
    blocks applied to exp'd scores (transposed, bf16). Blocks with other
    finite values become additive f32 blocks applied pre-exp (transposed).
    Returns (plan, mult_blocks, add_blocks); plan[i] is a list of
    (kt, uid_mult, uid_add).
    """
    ST = SEQ // P
    uniq_m, blocks_m = {}, []
    uniq_a, blocks_a = {}, []
    plan = []
    for i in range(ST):
        row = []
        for kt in range(ST):
            blk = mask[i * P : (i + 1) * P, kt * P : (kt + 1) * P]
            if (blk <= NEG_THRESH).all():
                continue
            if not blk.any():
                row.append((kt, -1, -1))
            elif ((blk == 0) | (blk <= NEG_THRESH)).all():
                key = blk.tobytes()
                if key not in uniq_m:
                    uniq_m[key] = len(blocks_m)
                    blocks_m.append(
                        np.ascontiguousarray((blk.T > NEG_THRESH).astype(np.float32))
                    )
                row.append((kt, uniq_m[key], -1))
            else:
                key = blk.tobytes()
                if key not in uniq_a:
                    uniq_a[key] = len(blocks_a)
                    blocks_a.append(np.ascontiguousarray(blk.T))
                row.append((kt, -1, uniq_a[key]))
        # fully masked query rows: leave empty; Aall is zero-filled for them
        plan.append(row)
    return plan, blocks_m, blocks_a


def make_rope_tables(cos_freq, sin_freq, SEQ, scale_quarter):
    """Build replicated [cos_rep (SEQ, NH*64) | sin_rep (SEQ, NH*64)] with
    sqrt(SCALE) folded in."""
    cos_t = np.tile(np.asarray(cos_freq, np.float32) * scale_quarter, (1, NH))
    sin_t = np.tile(np.asarray(sin_freq, np.float32) * scale_quarter, (1, NH))
    import ml_dtypes

    return np.ascontiguousarray(
        np.concatenate([cos_t, sin_t], axis=1).astype(ml_dtypes.bfloat16)
    )


_BUILD_CACHE = {}


def kernel(
    x,
    cos_freq,
    sin_freq,
    positions,
    mask,
    wq,
    wk,
    wv,
    wo,
    _trace=False,
):
    import sys

    if "/opt/trn_rl_repo" not in sys.path:
        sys.path.insert(0, "/opt/trn_rl_repo")
    from concourse.bass_utils import run_bass_kernel_spmd

    x = np.asarray(x, np.float32)
    mask = np.asarray(mask, np.float32)
    wq = np.asarray(wq, np.float32)
    wk = np.asarray(wk, np.float32)
    wv = np.asarray(wv, np.float32)
    wo = np.asarray(wo, np.float32)
    SEQ, DIM = x.shape
    assert wq.shape[0] == CORES * NH * D and wk.shape[0] == CORES * D
    assert 2 * SEQ == wq.shape[0], "flatten structure requires H*D == 2*SEQ"

    plan, blocks_m, blocks_a = analyze_mask(mask, SEQ)
    n_uniq, n_uniq_add = len(blocks_m), len(blocks_a)
    key = (SEQ, DIM, tuple(tuple(r) for r in plan))
    if key not in _BUILD_CACHE:
        _BUILD_CACHE[key] = build_attention_nc(SEQ, DIM, plan, n_uniq, n_uniq_add)
    nc = _BUILD_CACHE[key]

    import ml_dtypes

    bf16 = ml_dtypes.bfloat16
    f8 = ml_dtypes.float8_e4m3
    WSC = np.float32(64.0)  # weight pre-scale; undone via rope tables/V copy

    def f8hl(a):
        hi = a.astype(f8)
        lo = (a - hi.astype(np.float32)).astype(f8)
        return hi, lo

    # fold 1/64 into the rope tables (q and k both carry the x64 weights)
    scale_quarter = np.float32(D ** -0.25) / WSC
    cs = make_rope_tables(cos_freq, sin_freq, SEQ, scale_quarter)
    ST_, DD_ = SEQ // P, DIM // P
    xt = np.ascontiguousarray(x.reshape(ST_, P, DD_, P).transpose(3, 0, 2, 1))
    xh, xl = f8hl(xt)
    xT = np.ascontiguousarray(np.stack([xh, xl], axis=3))  # [p, st, t, 2, si]
    wot3 = np.ascontiguousarray(
        (WSC * wo.T).reshape(2 * SEQ // P, P, DIM).transpose(1, 0, 2)
    )  # [p, jt, m] = 64 * wo[m, jt*128+p]
    woh, wol = f8hl(wot3)
    JT_ = 2 * SEQ // P
    woT = np.ascontiguousarray(
        np.stack([woh, wol], axis=2)
        .reshape(P, JT_, 2, DIM // 512, 512)
        .transpose(0, 3, 1, 2, 4)
    )  # [p, mc, jt, hl, mi]
    if n_uniq:
        mbs = np.ascontiguousarray(np.stack(blocks_m, axis=0)).astype(bf16)
    else:
        mbs = np.zeros((1, P, P), bf16)
    if n_uniq_add:
        mbas = np.ascontiguousarray(np.stack(blocks_a, axis=0)).astype(np.float32)
    else:
        mbas = np.zeros((1, P, P), np.float32)

    in_maps = []
    for c in range(CORES):
        w_c = np.concatenate(
            [
                wq[c * NH * D : (c + 1) * NH * D],
                wk[c * D : (c + 1) * D],
                wv[c * D : (c + 1) * D],
            ],
            axis=0,
        )
        wt = np.ascontiguousarray(
            (WSC * w_c.T).reshape(DD_, P, -1).transpose(1, 0, 2)
        )  # [p, t, e] = 64 * w_c[e, t*128+p]
        wh, wl = f8hl(wt)
        whl = np.ascontiguousarray(np.stack([wh, wl], axis=2))
        in_maps.append(
            {
                "xT": xT,
                "wT": whl,
                "cs": cs,
                "maskb": mbs,
                "maskba": mbas,
                "woT": woT,
            }
        )

    import time as _time

    _t0 = _time.time()
    res = run_bass_kernel_spmd(nc, in_maps, list(range(CORES)), trace=_trace)
    global LAST_EXEC_NS
    LAST_EXEC_NS = int((_time.time() - _t0) * 1e9)
    outp = np.concatenate(
        [res.results[c]["out"] for c in range(CORES)], axis=0
    ).astype(np.float32)
    if _trace:
        return outp, res
    return outp



# revision 64
# speedup vs baseline: 1.0703x; 1.0034x over previous
"""Trainium2 Bass kernel for nn_Attention (GQA + RoPE + sliding-window mask).

Sharding: tensor-parallel over heads across 8 cores. Each core gets 4 q heads
and exactly 1 kv head (32 q / 8 kv heads, GQA group = 4). The reference's
quirky output flatten ((H,S,D)->(H,D,S)->reshape(S, H*D)) makes the final
projection contract over (d-parity, sequence) instead of heads, so the final
output is row-sharded by head block: core c produces rows [256c, 256c+256) of
the (2048, 4096) result with NO collective at all.

Per-core pipeline (all on one NeuronCore, same program on all 8 = pure SPMD):
  phase 1: QKV projections (bf16 matmuls) + RoPE (sqrt(scale) folded into the
           rope tables of both q and k) + DMA transposes into [d, s] layouts.
  phase 2: TRANSPOSED attention. Scores are computed as S^T[k, q] directly
           (K^T tile stationary, Q^T moving), so the exp'd probabilities land
           in SBUF already in the [k, q] layout PV needs - no P transposes.
           Softmax uses no running max (logits are O(10), exp biased by -8
           stays in range); denominators are per-q partition sums computed
           with free 1-wide ones-matmuls on the PE; causal masking is a 0/1
           triangular multiply on the bf16 P tile (DVE). PV then produces
           A[q, d] directly, normalized into the Aall layout by ACT.
  phase 3: final projection vs full wo (bf16), row slice out.
"""

import numpy as np
from contextlib import ExitStack

P = 128
D = 128  # head dim
NH = 4   # q heads per core
CORES = 8
NEG_THRESH = -1e8
EXP_BIAS = -8.0  # constant bias inside exp; cancels in normalization


def build_attention_nc(
    SEQ,
    DIM,
    plan,
    n_uniq,
    n_uniq_add=0,
):
    """Build the per-core Bass program.

    plan: list over q-tiles i (SEQ//128 entries) of lists of (kt, uid, uid_add)
          at 128x128 block granularity. uid == -1: no masking needed.
          uid >= 0: multiply the exp'd P tile by 0/1 block `uid` (DVE).
          uid_add >= 0: add f32 block `uid_add` to scores before exp (general
          additive masks; unused for causal). Blocks absent are fully masked.
    """
    import concourse.bass as bass
    import concourse.bacc as bacc
    import concourse.mybir as mybir
    import concourse.tile as tile

    f32 = mybir.dt.float32
    bf16 = mybir.dt.bfloat16

    ST = SEQ // P          # 16 s-tiles
    DD = DIM // P          # 32 contraction tiles
    EW = NH * D            # 512 q-projection width
    JT = 2 * SEQ // P      # 32 j-tiles for final matmul
    MC = DIM // 512        # 8 output chunks
    ITILES = (NH * 64) // P  # 2 output row tiles
    assert NH == 4 and SEQ % 512 == 0 and DIM % 512 == 0

    nc = bacc.Bacc(trn_type="TRN2", debug=False, num_devices=CORES)

    f8 = mybir.dt.float8e4

    # x and the QKV weights arrive as packed fp8 hi/lo pairs (hi = fp8(v),
    # lo = fp8(v - hi)); three DoubleRow matmuls per contraction-tile pair
    # compute hi*hi + lo*hi + hi*lo at 0.75x the bf16 cycle cost with ~2x
    # BETTER accuracy. Weights are host-scaled by 64 so the lo residuals
    # stay above fp8's subnormal floor; the 1/64 is folded into the rope
    # tables and the V copy.
    # xT[p, st, t, hl, si] = fp8hl(x[st*128+si, t*128+p])
    xT = nc.dram_tensor(
        "xT", [P, ST, DD, 2, P], f8, kind="ExternalInput"
    ).ap()
    # wT[p, t, hl, e] = fp8hl(64 * w_c[e, t*128+p])
    wT = nc.dram_tensor(
        "wT", [P, DD, 2, EW + 2 * D], f8, kind="ExternalInput"
    ).ap()
    cs = nc.dram_tensor("cs", [SEQ, EW], bf16, kind="ExternalInput").ap()
    mb = nc.dram_tensor(
        "maskb", [max(n_uniq, 1), P, P], bf16, kind="ExternalInput"
    ).ap()
    mba = nc.dram_tensor(
        "maskba", [max(n_uniq_add, 1), P, P], f32, kind="ExternalInput"
    ).ap()
    # woT[p, mc, jt, hl, mi] = fp8hl(64 * wo[mc*256+mi, jt*128+p]) -
    # chunk-major so each 256-wide chunk load is one contiguous run per
    # partition (full DMA rate)
    woT = nc.dram_tensor(
        "woT", [P, DIM // 512, JT, 2, 512], f8, kind="ExternalInput"
    ).ap()
    out = nc.dram_tensor("out", [NH * 64, DIM], bf16, kind="ExternalOutput").ap()

    with tile.TileContext(nc) as tc, ExitStack() as ctx:
        const = ctx.enter_context(tc.tile_pool(name="const", bufs=1))
        ones = const.tile([P, 1], bf16)
        nc.vector.memset(ones, 1.0)
        ebias = const.tile([P, 1], f32)
        nc.vector.memset(ebias, EXP_BIAS)
        # touch Exp at t=0 so the ACT table load doesn't stall phase 2
        scr = const.tile([P, 1], f32)
        nc.scalar.activation(
            out=scr, in_=ebias, func=mybir.ActivationFunctionType.Exp
        )
        inv64 = const.tile([P, 1], f32)
        nc.vector.memset(inv64, 1.0 / 64.0)

        pers = ctx.enter_context(tc.tile_pool(name="pers", bufs=1))
        QTt = pers.tile([P, NH, ST * P], bf16)   # [d, h, s]
        KTt = pers.tile([P, ST * P], bf16)       # [d, s]
        Vt = pers.tile([P, ST, D + 1], bf16)     # [k(part), ktile, d | ones]

        # wo streams in 8KB quarter-chunks ([mc, ddj-half, t-quarter]) into a
        # ring that lives for the whole kernel, so ~12 quarters prefetch into
        # phase 1's spare DMA bandwidth and phase 3 never stalls on wo loads.
        W3 = 512
        MC2 = DIM // 512
        wopool = ctx.enter_context(tc.tile_pool(name="wopool", bufs=13))
        wot_tiles = {}
        # consumption order within an mc: (ddj0,q0), (ddj1,q0), (ddj0,q1), (ddj1,q1)
        QUARTER_ORDER = ((0, 0), (1, 0), (0, 1), (1, 1))
        # all 32 quarters in consumption order; loads pop from the front so
        # arrival order always matches need order
        wo_queue = [
            (mc, dd, qq) for mc in range(MC2) for dd, qq in QUARTER_ORDER
        ]

        def load_next_wo(pool, n):
            for _ in range(n):
                if not wo_queue:
                    return
                mc, dd, qq = wo_queue.pop(0)
                wot = pool.tile(
                    [P, 8, 2, W3], f8, tag="wo", name=f"wot{mc}_{dd}_{qq}"
                )
                nc.sync.dma_start(
                    out=wot,
                    in_=woT[:, mc, dd * ST + qq * 8 : dd * ST + (qq + 1) * 8, :, :],
                )
                wot_tiles[(mc, dd, qq)] = wot

        # phase-1 prefetch: one quarter per phase-1b s-tile (mc 0-2
        # resident by phase-1 end) — one 3.2us transfer fits each s-tile's
        # spare DMA bandwidth without starving the next s-tile's x
        WO_PREFETCH_ST = tuple(range(4, 16))
        # col D is all-ones: PV's rhs [V | 1] also accumulates the softmax
        # denominator into the A psum's column D
        nc.vector.memset(Vt[:, :, D : D + 1], 1.0)
        if n_uniq > 0:
            mbt = pers.tile([P, n_uniq, P], bf16)
        if n_uniq_add > 0:
            mbat = pers.tile([P, n_uniq_add, P], f32)

        # ---------------- phase 1: projections + rope + layout ----------------
        with (
            tc.tile_pool(name="wpool", bufs=1) as wpool,
            tc.tile_pool(name="xpool", bufs=4) as xpool,
            tc.tile_pool(name="cspool", bufs=4) as cspool,
            tc.tile_pool(name="rpool", bufs=2) as rpool,
            tc.tile_pool(name="qps", bufs=4, space="PSUM") as qps,
            tc.tile_pool(name="kvps", bufs=4, space="PSUM") as kvps,
        ):
            wTt = wpool.tile([P, DD, 2, EW + 2 * D], f8)
            wTr = wT

            XG = min(8, DD)  # dd-tiles per streamed x chunk
            xTr = xT
            # Phase 1a streams the weights ONCE while consuming them
            # pair-major across the first NA s-tiles: the startup window is
            # pure DMA capacity (w 6.3MB + x), and s-tile-major order would
            # leave the PE idle ~half of it waiting for late weight pairs.
            NA = 4
            # Per-round x chunks for the NA s-tiles with the round's weight
            # pieces riding along. DMA emission is DEFERRED and woven into
            # the matmul emission below: the xT0 ring has 6 slots for 16
            # chunks, so a chunk's dma_start must be emitted only after the
            # matmuls reading its ring-predecessor exist, or the WAR is lost
            # and the transfer overwrites live data.
            xa = [[None] * (DD // 2) for _ in range(NA)]

            def emit_xa_dma(c, s):
                xTt = xpool.tile([P, XG, 2, P], f8, tag="xT0", bufs=6)
                nc.sync.dma_start(
                    out=xTt, in_=xTr[:, s, c * XG : (c + 1) * XG, :, :]
                )
                for tt in range(0, XG, 2):
                    xa[s][(c * XG + tt) // 2] = (c * XG + tt, xTt, tt)
                if c == 0:
                    # round 0: one weight pair after each x chunk, so the
                    # first pairs' operands land just-in-time
                    w0 = 2 * s
                    nc.sync.dma_start(
                        out=wTt[:, w0 : w0 + 2, :, :],
                        in_=wTr[:, w0 : w0 + 2, :, :],
                    )
                elif s in (0, 2):
                    w0 = c * XG + (s // 2) * (XG // 2)
                    nc.sync.dma_start(
                        out=wTt[:, w0 : w0 + XG // 2, :, :],
                        in_=wTr[:, w0 : w0 + XG // 2, :, :],
                    )

            def stream_x(st):
                chunks = []
                for g in range(DD // XG):
                    xTt = xpool.tile([P, XG, 2, P], f8, tag="xT")
                    nc.sync.dma_start(
                        out=xTt,
                        in_=xTr[:, st, g * XG : (g + 1) * XG, :, :],
                    )
                    chunks.extend(
                        (g * XG + tt, xTt, tt) for tt in range(0, XG, 2)
                    )
                return chunks

            DR = mybir.MatmulPerfMode.DoubleRow

            def mm_qkv(Qp, KVp, xTt, tt, t):
                # contraction pair (t, t+1): three DoubleRow terms. The two
                # hi-weight terms of BOTH psums come before the lo-weight
                # terms, so on the in-order PE queue the w-hi half of a
                # streamed weight pair enables 4 of 6 matmuls immediately.
                x_hh = xTt[:, tt : tt + 2, 0, :]
                x_ll = xTt[:, tt : tt + 2, 1, :]
                w_hh = wTt[:, t : t + 2, 0, 0:EW]
                w_ll = wTt[:, t : t + 2, 1, 0:EW]
                v_hh = wTt[:, t : t + 2, 0, EW : EW + 2 * D]
                v_ll = wTt[:, t : t + 2, 1, EW : EW + 2 * D]
                first, last = t == 0, t == DD - 2
                nc.tensor.matmul(
                    Qp, x_hh, w_hh, start=first, stop=False, perf_mode=DR
                )
                nc.tensor.matmul(
                    Qp, x_ll, w_hh, start=False, stop=False, perf_mode=DR
                )
                nc.tensor.matmul(
                    KVp, x_hh, v_hh, start=first, stop=False, perf_mode=DR
                )
                nc.tensor.matmul(
                    KVp, x_ll, v_hh, start=False, stop=False, perf_mode=DR
                )
                nc.tensor.matmul(
                    Qp, x_hh, w_ll, start=False, stop=last, perf_mode=DR
                )
                nc.tensor.matmul(
                    KVp, x_hh, v_ll, start=False, stop=last, perf_mode=DR
                )

            def mm_kv_only(KVp, xTt, tt, t):
                x_hh = xTt[:, tt : tt + 2, 0, :]
                x_ll = xTt[:, tt : tt + 2, 1, :]
                v_hh = wTt[:, t : t + 2, 0, EW : EW + 2 * D]
                v_ll = wTt[:, t : t + 2, 1, EW : EW + 2 * D]
                first, last = t == 0, t == DD - 2
                nc.tensor.matmul(
                    KVp, x_hh, v_hh, start=first, stop=False, perf_mode=DR
                )
                nc.tensor.matmul(
                    KVp, x_ll, v_hh, start=False, stop=False, perf_mode=DR
                )
                nc.tensor.matmul(
                    KVp, x_hh, v_ll, start=False, stop=last, perf_mode=DR
                )

            def mm_q_only(Qp, xTt, tt, t):
                x_hh = xTt[:, tt : tt + 2, 0, :]
                x_ll = xTt[:, tt : tt + 2, 1, :]
                w_hh = wTt[:, t : t + 2, 0, 0:EW]
                w_ll = wTt[:, t : t + 2, 1, 0:EW]
                first, last = t == 0, t == DD - 2
                nc.tensor.matmul(
                    Qp, x_hh, w_hh, start=first, stop=False, perf_mode=DR
                )
                nc.tensor.matmul(
                    Qp, x_ll, w_hh, start=False, stop=False, perf_mode=DR
                )
                nc.tensor.matmul(
                    Qp, x_hh, w_ll, start=False, stop=last, perf_mode=DR
                )

            # rope via strided even/odd halves (2-level APs only - 3-level
            # APs overflow the fixed ISA instruction encoding).
            def ttr_ew(out, in0, in1, op):
                nc.vector.tensor_tensor(out=out, in0=in0, in1=in1, op=op)

            A_ = mybir.AluOpType
            HF = EW // 2  # 256: cos table width for q

            def rope_k(st_, KVp_, cst_):
                t1 = rpool.tile([P, D // 2], f32, tag="t1")
                t2 = rpool.tile([P, D // 2], f32, tag="t2")
                rk = rpool.tile([P, D], bf16, tag="rk")
                k_ev, k_od = KVp_[:, 0:D:2], KVp_[:, 1:D:2]
                cosk, sink = cst_[:, 0 : D // 2], cst_[:, HF : HF + D // 2]
                ttr_ew(t1, k_ev, cosk, A_.mult)
                ttr_ew(t2, k_od, sink, A_.mult)
                ttr_ew(rk[:, 0:D:2], t1, t2, A_.subtract)
                ttr_ew(t1, k_ev, sink, A_.mult)
                ttr_ew(t2, k_od, cosk, A_.mult)
                ttr_ew(rk[:, 1:D:2], t1, t2, A_.add)

                # V -> bf16 [k, d] layout (ACT copy, cast, undo the x64
                # weight scaling)
                nc.scalar.activation(
                    out=Vt[:, st_, 0:D],
                    in_=KVp_[:, D : 2 * D],
                    func=mybir.ActivationFunctionType.Copy,
                    scale=inv64,
                )
                nc.sync.dma_start_transpose(
                    out=KTt[:, st_ * P : (st_ + 1) * P], in_=rk
                )

            def rope_q(st_, Qp_, cst_):
                rq = rpool.tile([P, EW], bf16, tag="rq")
                t1 = rpool.tile([P, HF], f32, tag="t1")
                t2 = rpool.tile([P, HF], f32, tag="t2")
                cosr, sinr = cst_[:, 0:HF], cst_[:, HF : 2 * HF]
                q_ev, q_od = Qp_[:, 0:EW:2], Qp_[:, 1:EW:2]
                ttr_ew(t1, q_ev, cosr, A_.mult)
                ttr_ew(t2, q_od, sinr, A_.mult)
                ttr_ew(rq[:, 0:EW:2], t1, t2, A_.subtract)
                ttr_ew(t1, q_ev, sinr, A_.mult)
                ttr_ew(t2, q_od, cosr, A_.mult)
                ttr_ew(rq[:, 1:EW:2], t1, t2, A_.add)

                # transpose rq (per head) into [d, s] via the DMA
                # transpose engine (keeps PE free for matmuls)
                nc.sync.dma_start_transpose(
                    out=QTt[:, :, st_ * P : (st_ + 1) * P], in_=rq
                )

            # ---- phase 1a: s-tiles 0..NA-1 pair-major vs the streaming w:
            # each weight pair is consumed against NA s-tiles as it lands,
            # so the PE tracks the DMA-capacity-bound startup window instead
            # of idling for late pairs
            Qpa = [
                qps.tile([P, EW], f32, tag="Qp", name=f"Qpa{s}")
                for s in range(NA)
            ]
            KVpa = [
                kvps.tile([P, 2 * D], f32, tag="KVp", name=f"KVpa{s}")
                for s in range(NA)
            ]
            # fresh xT0 slots: round 0 + half of round 1
            for s in range(NA):
                emit_xa_dma(0, s)
            emit_xa_dma(1, 0)
            emit_xa_dma(1, 1)
            # masks are tiny; land them long before phase 2 needs them
            if n_uniq > 0:
                nc.sync.dma_start(out=mbt, in_=mb.rearrange("u p m -> p u m"))
            if n_uniq_add > 0:
                nc.sync.dma_start(out=mbat, in_=mba.rearrange("u p m -> p u m"))
            pending_dma = [(1, 2), (1, 3)] + [
                (c, s) for c in (2, 3) for s in range(NA)
            ]
            for rnd in range(DD // XG):
                for pr in range(rnd * 4, rnd * 4 + 4):
                    for s in range(NA):
                        t, xTt, tt = xa[s][pr]
                        mm_qkv(Qpa[s], KVpa[s], xTt, tt, t)
                # ring slots of the next chunks now have their readers
                # emitted; release the next DMAs
                for _ in range(4 if rnd < 2 else 2):
                    if pending_dma:
                        emit_xa_dma(*pending_dma.pop(0))
            cs_a = []
            for s in range(NA):
                csa = cspool.tile([P, EW], bf16, tag="cs")
                nc.sync.dma_start(out=csa, in_=cs[s * P : (s + 1) * P, :])
                cs_a.append(csa)
            for s in range(NA):
                rope_k(s, KVpa[s], cs_a[s])
                rope_q(s, Qpa[s], cs_a[s])

            # ---- phase 1b: remaining s-tiles, s-tile-major ----
            wo_pending = [False]

            def flush_wo_pending():
                if wo_pending[0]:
                    wo_pending[0] = False
                    load_next_wo(wopool, 1)

            for st in range(NA, ST):
                cst = cspool.tile([P, EW], bf16, tag="cs")
                nc.sync.dma_start(out=cst, in_=cs[st * P : (st + 1) * P, :])

                if st >= ST - 3:
                    # last s-tiles: run the whole KV contraction first and
                    # rope/transpose K before the Q matmuls, so the K tiles
                    # phase 2's first score chunks need are ready ~5us
                    # earlier (the K transpose rides the in-order DMA queue)
                    Qp = qps.tile([P, EW], f32, tag="Qp")
                    KVp = kvps.tile([P, 2 * D], f32, tag="KVp")
                    chunks = stream_x(st)
                    flush_wo_pending()
                    for t, xTt, tt in chunks:
                        mm_kv_only(KVp, xTt, tt, t)
                    rope_k(st, KVp, cst)
                    for t, xTt, tt in chunks:
                        mm_q_only(Qp, xTt, tt, t)
                    later = [(st, Qp, None, cst)]
                else:
                    Qp = qps.tile([P, EW], f32, tag="Qp")
                    KVp = kvps.tile([P, 2 * D], f32, tag="KVp")
                    for t, xTt, tt in stream_x(st):
                        mm_qkv(Qp, KVp, xTt, tt, t)
                    flush_wo_pending()
                    later = [(st, Qp, KVp, cst)]

                for st_, Qp_, KVp_, cst_ in later:
                    if KVp_ is not None:
                        rope_k(st_, KVp_, cst_)
                    rope_q(st_, Qp_, cst_)

                if st in WO_PREFETCH_ST:
                    # deferred: emitted after the NEXT s-tile's x chunks so
                    # the 3.2us transfer never delays them on the queue
                    wo_pending[0] = True
            flush_wo_pending()

        # ---------------- phase 2: attention (transposed scores) --------------
        apool = ctx.enter_context(tc.tile_pool(name="apool", bufs=1))
        # split by head-pair so phase 3's first row-tile can start once
        # heads 0-1 finish, overlapping the rest of phase 2. A is stored as
        # fp8 hi/lo pairs for the compensated-fp8 output projection.
        Aall = [
            apool.tile([P, 2 * ST * D], f8, name=f"Aall{i}")
            for i in range(NH // 2)
        ]
        Aallr = [
            apool.tile([P, 2 * ST * D], f8, name=f"Aallr{i}")
            for i in range(NH // 2)
        ]
        # PSUM pool order matters: pools opened first reuse phase 1's freed
        # qps/kvps bytes and inherit a WAR on the last s-tile's rope reads.
        # ops (phase 3) and aps/dsps (needed a few steps into phase 2) absorb
        # that; sps (needed immediately) lands on fresh bytes.
        with (
            tc.tile_pool(name="ops", bufs=2, space="PSUM") as ops,
            tc.tile_pool(name="aps", bufs=2, space="PSUM") as aps,
            tc.tile_pool(name="sps", bufs=2, space="PSUM") as sps,
            tc.tile_pool(name="ptsb", bufs=4) as ptsb,
            tc.tile_pool(name="stat", bufs=8) as stat,
            tc.tile_pool(name="osb", bufs=4) as osb,
            tc.tile_pool(name="wopoolB", bufs=5) as wopoolB,
        ):
            # q-tiles processed long-rows-first so the front/back pipeline
            # fills with real work instead of stalling on semaphore chains
            # through tiny rows at phase-2 entry. Head 0 starts at i=12 (not
            # 15) because q-tile 15's transpose is still draining through the
            # DMA queue when phase 2 begins.
            steps = []
            h0_order = [10, 11, 12, 13, 14, 15] + list(range(9, -1, -1))
            for h in range(NH):
                for i in (h0_order if h == 0 else reversed(range(ST))):
                    if plan[i]:
                        steps.append((h, i))

            # per-(h, qs) psum tiles holding 4 query-tiles' worth of slots;
            # accumulation groups are time-sequential so sharing one 2KB
            # zero-region is safe (earlier slots are only read afterwards)
            blk_tiles = {}

            def emit_front(step):
                """Scores (PE) + exp (ACT) + causal 0/1 multiply (DVE).

                8-block exp chunks (2-bank PSUM S tiles): ACT instruction
                count halves, and ACT's ~185ns per-instruction access
                latency is what makes phase 2 ACT-critical."""
                h, i = step
                row = plan[i]
                PTt = ptsb.tile([P, ST, P], bf16, tag="PT")
                for c0 in range(0, len(row), 8):
                    chunk = row[c0 : c0 + 8]
                    S = sps.tile([P, 1024], f32, tag="S")
                    for j, (kt, uid, uida) in enumerate(chunk):
                        nc.tensor.matmul(
                            S[:, j * P : (j + 1) * P],
                            KTt[:, kt * P : (kt + 1) * P],
                            QTt[:, h, i * P : (i + 1) * P],
                            start=True,
                            stop=True,
                        )
                        if uida >= 0:
                            nc.vector.tensor_add(
                                S[:, j * P : (j + 1) * P],
                                S[:, j * P : (j + 1) * P],
                                mbat[:, uida, :],
                            )
                    nc.scalar.activation(
                        out=PTt[:, c0 : c0 + len(chunk), :],
                        in_=S[:, 0 : len(chunk) * P],
                        func=mybir.ActivationFunctionType.Exp,
                        bias=ebias,
                    )
                    for j, (kt, uid, uida) in enumerate(chunk):
                        if uid >= 0:
                            nc.vector.tensor_tensor(
                                out=PTt[:, c0 + j, :],
                                in0=PTt[:, c0 + j, :],
                                in1=mbt[:, uid, :],
                                op=mybir.AluOpType.mult,
                            )
                return PTt

            def emit_back(step, PTt):
                """PV with fused denominator (PE) + recip (DVE) + normalized
                fp8 hi/lo Aall writes (DVE)."""
                h, i = step
                row = plan[i]
                qs, qi = i // 4, i % 4
                A = aps.tile([P, D + 1], f32, tag="A")
                nkt = len(row)
                for n, (kt, uid, uida) in enumerate(row):
                    nc.tensor.matmul(
                        A,
                        PTt[:, n, :],
                        Vt[:, kt, :],
                        start=(n == 0),
                        stop=(n == nkt - 1),
                    )
                rec = stat.tile([P, 1], f32, tag="rec")
                nc.vector.reciprocal(rec, A[:, D : D + 1])
                # Aall layout: [sp, (t*2 + dd)*128 + hb*64 + p] so the final
                # matmul's stationary slices are contiguous (walrus requires
                # a single free dim on weight APs)
                hb = h % 2

                def dv(Ah):
                    # dview[sp, p, dd] == Ah[:, i*256 + dd*128 + hb*64 + p]
                    return Ah[:, i * 2 * P : (i + 1) * 2 * P].rearrange(
                        "a (dd j) -> a dd j", dd=2
                    )[:, :, hb * 64 : hb * 64 + 64].rearrange(
                        "a dd p -> a p dd"
                    )

                dhi, dlo = dv(Aall[h // 2]), dv(Aallr[h // 2])
                Asl = A[:, 0:D].rearrange(
                    "a (p two) -> a p two", two=2
                )
                nc.vector.tensor_scalar_mul(dhi, Asl, rec)
                # lo = A*rec - hi (both fp8 rounded by the output dtype)
                nc.vector.scalar_tensor_tensor(
                    out=dlo,
                    in0=Asl,
                    scalar=rec,
                    in1=dhi,
                    op0=mybir.AluOpType.mult,
                    op1=mybir.AluOpType.subtract,
                )

            # zero Aall regions for fully-masked query rows (unreachable for
            # causal masks, but keeps the flatten well-defined). Emitted
            # before any phase-3 matmul can read them.
            for i in range(ST):
                if not plan[i]:
                    for h in range(NH):
                        for Ah in (Aall[h // 2], Aallr[h // 2]):
                            nc.vector.memset(
                                Ah[:, i * 2 * P : (i + 1) * 2 * P], 0.0
                            )

            # ---------------- phase 3 (interleaved into phase 2) -----------
            # Phase 2 is ACT(exp)-throughput-bound, leaving the PE with idle
            # slack between steps; phase-3 matmuls are drip-fed into that
            # slack as soon as their Aall inputs are final. wo quarters for
            # mc 0-2 prefetched during phase 1; the rest ride a close-
            # triggered ring (close of (mc, 1) frees mc's 4 quarters → load
            # mc+3's), so the in-order SP queue never blocks on a WAR wait.
            p3_queue = []  # (mc, it, u) units in emission order
            p3_open = {}
            p3_cnt = {}  # units emitted per psum key; drives start/stop
            pushed = set()

            # the very last block runs as four 128-wide quarter-accumulations
            # so each quarter's copy/store overlaps the next one's matmuls
            # and the post-last-matmul drain chain is a single 128-col piece
            FINAL = (MC2 - 1, 1)

            def close_p3_block(mc, it, half):
                O = p3_open.pop((mc, it, half))
                if (mc, it) == (3, 1):
                    # (3,1) closing frees wopoolB's mc3 slots + wopool's
                    # spare; later closes free wopool's mc0-2 slots in ring
                    # order, so pops always land where the WAR clears next
                    load_next_wo(wopoolB, 3)
                    load_next_wo(wopool, 1)
                elif it == 1 and half in (None, 3):
                    load_next_wo(wopool, 4)
                base = mc * W3 + (half * 128 if half is not None else 0)
                width = 128 if half is not None else W3
                npc = 1
                w = width // npc
                for pc in range(npc):
                    Ot = osb.tile([P, w], bf16, tag="Ot")
                    nc.scalar.activation(
                        out=Ot,
                        in_=O[:, pc * w : (pc + 1) * w],
                        func=mybir.ActivationFunctionType.Copy,
                        scale=inv64,
                    )
                    nc.sync.dma_start(
                        out=out[
                            it * P : (it + 1) * P,
                            base + pc * w : base + (pc + 1) * w,
                        ],
                        in_=Ot,
                    )

            NU = 3 * JT // 2  # 48 DoubleRow units per block

            def pair_ap(Ah, ddj, t):
                idx = t * 2 + ddj
                return Ah.rearrange("a (tt j) -> a tt j", j=P)[
                    :, idx : idx + 3 : 2, :
                ]

            def push_block(mc, it):
                pushed.add((mc, it))
                n = 4 * NU if (mc, it) == FINAL else NU
                p3_queue.extend([(mc, it, u) for u in range(n)])

            def emit_p3(budget):
                emitted = 0
                while p3_queue and emitted < budget:
                    mc, it, u = p3_queue.pop(0)
                    if (mc, it) == FINAL:
                        half, uu = divmod(u, NU)
                    else:
                        half, uu = None, u
                    key = (mc, it, half)
                    if key not in p3_open:
                        p3_open[key] = ops.tile(
                            [P, 128 if half is not None else 512],
                            f32,
                            tag="O",
                            name=f"O{mc}_{it}_{half}",
                        )
                    O = p3_open[key]
                    pi, term = uu // 3, uu % 3
                    t, ddj = 2 * (pi // 2), pi % 2
                    lhsT = pair_ap(
                        (Aall if term != 1 else Aallr)[it], ddj, t
                    )
                    rhs = wot_tiles[(mc, ddj, t // 8)][
                        :, t % 8 : t % 8 + 2, 1 if term == 2 else 0, :
                    ]
                    if half is not None:
                        rhs = rhs[:, :, half * 128 : (half + 1) * 128]
                    # drip-fed units arrive out of unit-id order, so the
                    # psum group's start (zeroing) / stop must track the
                    # EMISSION count, not the unit id
                    cnt = p3_cnt.get(key, 0)
                    p3_cnt[key] = cnt + 1
                    nc.tensor.matmul(
                        O,
                        lhsT,
                        rhs,
                        start=(cnt == 0),
                        stop=(cnt == NU - 1),
                        perf_mode=DR,
                    )
                    emitted += 1
                    if cnt == NU - 1:
                        close_p3_block(mc, it, half)
                return emitted

            # stage the next 5 quarters (3_0_0..4_0_0) into the spare wopool
            # slot + the fresh wopoolB ring right at phase-2 start: fresh
            # slots have no WAR, so these transfers run in phase 2's
            # otherwise-idle DMA window
            load_next_wo(wopool, 1)
            load_next_wo(wopoolB, 5)

            # Deep software pipeline: PE runs step n's scores while ACT/DVE
            # finish earlier steps, so the PE never waits on exp results
            DEPTH = 3
            pending = []

            all_rows = all(plan[i] for i in range(ST))

            steps_h1 = [s for s in steps if s[0] == 1]

            def after_back(s0, front_step):
                h0_, i0_ = s0
                if steps_h1 and s0 == steps_h1[-1]:
                    # ALL of head 0-1's PV writes are now emitted: (1,0)'s
                    # units may be queued without reading not-yet-written
                    # Aall[0] rows (pushing at head-2's first FRONT would
                    # race the last DEPTH backs of head 1)
                    if not all_rows and (0, 0) not in pushed:
                        push_block(0, 0)
                    push_block(1, 0)
                if h0_ == 1 and all_rows and i0_ % 2 == 0:
                    # with descending q-tile order, rows (i0, i0+1) are both
                    # final once head 1 reaches even i0; drip block (0,0)'s
                    # matching jt-pair units in right here
                    pushed.add((0, 0))
                    for pi in (i0_, i0_ + 1):
                        p3_queue.extend(
                            [(0, 0, 3 * pi + tm) for tm in range(3)]
                        )
                if h0_ == 3 and all_rows and i0_ % 2 == 0:
                    # same for (0,1) as head 3 completes its rows: closing
                    # (0,1) at the very start of the tail frees mc0's wopool
                    # slots, so the mc4-7 load chain starts ~5us earlier
                    pushed.add((0, 1))
                    for pi in (i0_, i0_ + 1):
                        p3_queue.extend(
                            [(0, 1, 3 * pi + tm) for tm in range(3)]
                        )
                # budget ~ the ACT-over-PE slack of the step the PE is
                # currently chewing on
                nch = (len(plan[front_step[1]]) + 3) // 4 if front_step else 2
                emit_p3(max(3, min(9, 2 * nch + 3)))

            # blocks (1,0) and (2,0) become ready when heads 0-1 are done
            # (mc 0-2 wo quarters are phase-1-prefetched; mc 3+ stay in the
            # tail where the close-triggered ring covers them)
            steps_h2 = [s for s in steps if s[0] == 2]
            steps_h3 = [s for s in steps if s[0] == 3]
            for step in steps:
                if steps_h3 and step == steps_h3[0]:
                    push_block(2, 0)
                    push_block(3, 0)
                PTt = emit_front(step)
                pending.append((step, PTt))
                if len(pending) > DEPTH:
                    s0, p0 = pending.pop(0)
                    emit_back(s0, p0)
                    after_back(s0, step)
            for s0, p0 in pending:
                emit_back(s0, p0)
                after_back(s0, None)

            # remaining blocks: it=1 of mc0-3 first — their closes free the
            # wopool/wopoolB slots for mc4-7's ring loads in consumption
            # order ((0,1) leftovers drain first from the head-3 drip)
            base_rest = [(0, 1), (1, 1), (2, 1), (3, 1), (4, 0), (4, 1),
                         (5, 0), (5, 1), (6, 0), (6, 1), (7, 0), (7, 1),
                         (3, 0)]
            for mc, it in base_rest:
                if (mc, it) not in pushed:
                    push_block(mc, it)
            for mc in range(MC2):
                for it in range(ITILES):
                    if (mc, it) not in pushed:
                        push_block(mc, it)
            emit_p3(10 ** 9)

    nc.compile()
    return nc


def analyze_mask(mask, SEQ):
    """Classify 128x128 mask blocks: skip / free / masked.

    Masked blocks that only contain {0, -inf-ish} become 0/1 multiplicative
    blocks applied to exp'd scores (transposed, bf16). Blocks with other
    finite values become additive f32 blocks applied pre-exp (transposed).
    Returns (plan, mult_blocks, add_blocks); plan[i] is a list of
    (kt, uid_mult, uid_add).
    """
    ST = SEQ // P
    uniq_m, blocks_m = {}, []
    uniq_a, blocks_a = {}, []
    plan = []
    for i in range(ST):
        row = []
        for kt in range(ST):
            blk = mask[i * P : (i + 1) * P, kt * P : (kt + 1) * P]
            if (blk <= NEG_THRESH).all():
                continue
            if not blk.any():
                row.append((kt, -1, -1))
            elif ((blk == 0) | (blk <= NEG_THRESH)).all():
                key = blk.tobytes()
                if key not in uniq_m:
                    uniq_m[key] = len(blocks_m)
                    blocks_m.append(
                        np.ascontiguousarray((blk.T > NEG_THRESH).astype(np.float32))
                    )
                row.append((kt, uniq_m[key], -1))
            else:
                key = blk.tobytes()
                if key not in uniq_a:
                    uniq_a[key] = len(blocks_a)
                    blocks_a.append(np.ascontiguousarray(blk.T))
                row.append((kt, -1, uniq_a[key]))
        # fully masked query rows: leave empty; Aall is zero-filled for them
        plan.append(row)
    return plan, blocks_m, blocks_a


def make_rope_tables(cos_freq, sin_freq, SEQ, scale_quarter):
    """Build replicated [cos_rep (SEQ, NH*64) | sin_rep (SEQ, NH*64)] with
    sqrt(SCALE) folded in."""
    cos_t = np.tile(np.asarray(cos_freq, np.float32) * scale_quarter, (1, NH))
    sin_t = np.tile(np.asarray(sin_freq, np.float32) * scale_quarter, (1, NH))
    import ml_dtypes

    return np.ascontiguousarray(
        np.concatenate([cos_t, sin_t], axis=1).astype(ml_dtypes.bfloat16)
    )


_BUILD_CACHE = {}


def kernel(
    x,
    cos_freq,
    sin_freq,
    positions,
    mask,
    wq,
    wk,
    wv,
    wo,
    _trace=False,
):
    import sys

    if "/opt/trn_rl_repo" not in sys.path:
        sys.path.insert(0, "/opt/trn_rl_repo")
    from concourse.bass_utils import run_bass_kernel_spmd

    x = np.asarray(x, np.float32)
    mask = np.asarray(mask, np.float32)
    wq = np.asarray(wq, np.float32)
    wk = np.asarray(wk, np.float32)
    wv = np.asarray(wv, np.float32)
    wo = np.asarray(wo, np.float32)
    SEQ, DIM = x.shape
    assert wq.shape[0] == CORES * NH * D and wk.shape[0] == CORES * D
    assert 2 * SEQ == wq.shape[0], "flatten structure requires H*D == 2*SEQ"

    plan, blocks_m, blocks_a = analyze_mask(mask, SEQ)
    n_uniq, n_uniq_add = len(blocks_m), len(blocks_a)
    key = (SEQ, DIM, tuple(tuple(r) for r in plan))
    if key not in _BUILD_CACHE:
        _BUILD_CACHE[key] = build_attention_nc(SEQ, DIM, plan, n_uniq, n_uniq_add)
    nc = _BUILD_CACHE[key]

    import ml_dtypes

    bf16 = ml_dtypes.bfloat16
    f8 = ml_dtypes.float8_e4m3
    WSC = np.float32(64.0)  # weight pre-scale; undone via rope tables/V copy

    def f8hl(a):
        hi = a.astype(f8)
        lo = (a - hi.astype(np.float32)).astype(f8)
        return hi, lo

    # fold 1/64 into the rope tables (q and k both carry the x64 weights)
    scale_quarter = np.float32(D ** -0.25) / WSC
    cs = make_rope_tables(cos_freq, sin_freq, SEQ, scale_quarter)
    ST_, DD_ = SEQ // P, DIM // P
    xt = np.ascontiguousarray(x.reshape(ST_, P, DD_, P).transpose(3, 0, 2, 1))
    xh, xl = f8hl(xt)
    xT = np.ascontiguousarray(np.stack([xh, xl], axis=3))  # [p, st, t, 2, si]
    wot3 = np.ascontiguousarray(
        (WSC * wo.T).reshape(2 * SEQ // P, P, DIM).transpose(1, 0, 2)
    )  # [p, jt, m] = 64 * wo[m, jt*128+p]
    woh, wol = f8hl(wot3)
    JT_ = 2 * SEQ // P
    woT = np.ascontiguousarray(
        np.stack([woh, wol], axis=2)
        .reshape(P, JT_, 2, DIM // 512, 512)
        .transpose(0, 3, 1, 2, 4)
    )  # [p, mc, jt, hl, mi]
    if n_uniq:
        mbs = np.ascontiguousarray(np.stack(blocks_m, axis=0)).astype(bf16)
    else:
        mbs = np.zeros((1, P, P), bf16)
    if n_uniq_add:
        mbas = np.ascontiguousarray(np.stack(blocks_a, axis=0)).astype(np.float32)
    else:
        mbas = np.zeros((1, P, P), np.float32)

    in_maps = []
    for c in range(CORES):
        w_c = np.concatenate(
            [
                wq[c * NH * D : (c + 1) * NH * D],
                wk[c * D : (c + 1) * D],
                wv[c * D : (c + 1) * D],
            ],
            axis=0,
        )
        wt = np.ascontiguousarray(
            (WSC * w_c.T).reshape(DD_, P, -1).transpose(1, 0, 2)
        )  # [p, t, e] = 64 * w_c[e, t*128+p]
        wh, wl = f8hl(wt)
        whl = np.ascontiguousarray(np.stack([wh, wl], axis=2))
        in_maps.append(
            {
                "xT": xT,
                "wT": whl,
                "cs": cs,
                "maskb": mbs,
                "maskba": mbas,
                "woT": woT,
            }
        )

    import time as _time

    _t0 = _time.time()
    res = run_bass_kernel_spmd(nc, in_maps, list(range(CORES)), trace=_trace)
    global LAST_EXEC_NS
    LAST_EXEC_NS = int((_time.time() - _t0) * 1e9)
    outp = np.concatenate(
        [res.results[c]["out"] for c in range(CORES)], axis=0
    ).astype(np.float32)
    if _trace:
        return outp, res
    return outp



# revision 69
# speedup vs baseline: 1.0759x; 1.0052x over previous
"""Trainium2 Bass kernel for nn_Attention (GQA + RoPE + sliding-window mask).

Sharding: tensor-parallel over heads across 8 cores. Each core gets 4 q heads
and exactly 1 kv head (32 q / 8 kv heads, GQA group = 4). The reference's
quirky output flatten ((H,S,D)->(H,D,S)->reshape(S, H*D)) makes the final
projection contract over (d-parity, sequence) instead of heads, so the final
output is row-sharded by head block: core c produces rows [256c, 256c+256) of
the (2048, 4096) result with NO collective at all.

Per-core pipeline (all on one NeuronCore, same program on all 8 = pure SPMD):
  phase 1: QKV projections (bf16 matmuls) + RoPE (sqrt(scale) folded into the
           rope tables of both q and k) + DMA transposes into [d, s] layouts.
  phase 2: TRANSPOSED attention. Scores are computed as S^T[k, q] directly
           (K^T tile stationary, Q^T moving), so the exp'd probabilities land
           in SBUF already in the [k, q] layout PV needs - no P transposes.
           Softmax uses no running max (logits are O(10), exp biased by -8
           stays in range); denominators are per-q partition sums computed
           with free 1-wide ones-matmuls on the PE; causal masking is a 0/1
           triangular multiply on the bf16 P tile (DVE). PV then produces
           A[q, d] directly, normalized into the Aall layout by ACT.
  phase 3: final projection vs full wo (bf16), row slice out.
"""

import numpy as np
from contextlib import ExitStack

P = 128
D = 128  # head dim
NH = 4   # q heads per core
CORES = 8
NEG_THRESH = -1e8
EXP_BIAS = -8.0  # constant bias inside exp; cancels in normalization


def build_attention_nc(
    SEQ,
    DIM,
    plan,
    n_uniq,
    n_uniq_add=0,
):
    """Build the per-core Bass program.

    plan: list over q-tiles i (SEQ//128 entries) of lists of (kt, uid, uid_add)
          at 128x128 block granularity. uid == -1: no masking needed.
          uid >= 0: multiply the exp'd P tile by 0/1 block `uid` (DVE).
          uid_add >= 0: add f32 block `uid_add` to scores before exp (general
          additive masks; unused for causal). Blocks absent are fully masked.
    """
    import concourse.bass as bass
    import concourse.bacc as bacc
    import concourse.mybir as mybir
    import concourse.tile as tile

    f32 = mybir.dt.float32
    bf16 = mybir.dt.bfloat16

    ST = SEQ // P          # 16 s-tiles
    DD = DIM // P          # 32 contraction tiles
    EW = NH * D            # 512 q-projection width
    JT = 2 * SEQ // P      # 32 j-tiles for final matmul
    MC = DIM // 512        # 8 output chunks
    ITILES = (NH * 64) // P  # 2 output row tiles
    assert NH == 4 and SEQ % 512 == 0 and DIM % 512 == 0

    nc = bacc.Bacc(trn_type="TRN2", debug=False, num_devices=CORES)

    f8 = mybir.dt.float8e4

    # x and the QKV weights arrive as packed fp8 hi/lo pairs (hi = fp8(v),
    # lo = fp8(v - hi)); three DoubleRow matmuls per contraction-tile pair
    # compute hi*hi + lo*hi + hi*lo at 0.75x the bf16 cycle cost with ~2x
    # BETTER accuracy. Weights are host-scaled by 64 so the lo residuals
    # stay above fp8's subnormal floor; the 1/64 is folded into the rope
    # tables and the V copy.
    # xT[p, st, t, hl, si] = fp8hl(x[st*128+si, t*128+p])
    xT = nc.dram_tensor(
        "xT", [P, ST, DD, 2, P], f8, kind="ExternalInput"
    ).ap()
    # wT[p, t, hl, e] = fp8hl(64 * w_c[e, t*128+p])
    wT = nc.dram_tensor(
        "wT", [P, DD, 2, EW + 2 * D], f8, kind="ExternalInput"
    ).ap()
    cs = nc.dram_tensor("cs", [SEQ, EW], bf16, kind="ExternalInput").ap()
    mb = nc.dram_tensor(
        "maskb", [max(n_uniq, 1), P, P], bf16, kind="ExternalInput"
    ).ap()
    mba = nc.dram_tensor(
        "maskba", [max(n_uniq_add, 1), P, P], f32, kind="ExternalInput"
    ).ap()
    # woT[p, mc, jt, hl, mi] = fp8hl(64 * wo[mc*256+mi, jt*128+p]) -
    # chunk-major so each 256-wide chunk load is one contiguous run per
    # partition (full DMA rate)
    woT = nc.dram_tensor(
        "woT", [P, DIM // 512, JT, 2, 512], f8, kind="ExternalInput"
    ).ap()
    out = nc.dram_tensor("out", [NH * 64, DIM], bf16, kind="ExternalOutput").ap()

    with tile.TileContext(nc) as tc, ExitStack() as ctx:
        const = ctx.enter_context(tc.tile_pool(name="const", bufs=1))
        ones = const.tile([P, 1], bf16)
        nc.vector.memset(ones, 1.0)
        ebias = const.tile([P, 1], f32)
        nc.vector.memset(ebias, EXP_BIAS)
        # touch Exp at t=0 so the ACT table load doesn't stall phase 2
        scr = const.tile([P, 1], f32)
        nc.scalar.activation(
            out=scr, in_=ebias, func=mybir.ActivationFunctionType.Exp
        )
        inv64 = const.tile([P, 1], f32)
        nc.vector.memset(inv64, 1.0 / 64.0)

        pers = ctx.enter_context(tc.tile_pool(name="pers", bufs=1))
        QTt = pers.tile([P, NH, ST * P], bf16)   # [d, h, s]
        KTt = pers.tile([P, ST * P], bf16)       # [d, s]
        Vt = pers.tile([P, ST, D + 1], bf16)     # [k(part), ktile, d | ones]

        # wo streams in 8KB quarter-chunks ([mc, ddj-half, t-quarter]) into a
        # ring that lives for the whole kernel, so ~12 quarters prefetch into
        # phase 1's spare DMA bandwidth and phase 3 never stalls on wo loads.
        W3 = 512
        MC2 = DIM // 512
        wopool = ctx.enter_context(tc.tile_pool(name="wopool", bufs=13))
        wot_tiles = {}
        # consumption order within an mc: (ddj0,q0), (ddj1,q0), (ddj0,q1), (ddj1,q1)
        QUARTER_ORDER = ((0, 0), (1, 0), (0, 1), (1, 1))
        # all 32 quarters in consumption order; loads pop from the front so
        # arrival order always matches need order
        wo_queue = [
            (mc, dd, qq) for mc in range(MC2) for dd, qq in QUARTER_ORDER
        ]

        def load_next_wo(pool, n):
            for _ in range(n):
                if not wo_queue:
                    return
                mc, dd, qq = wo_queue.pop(0)
                wot = pool.tile(
                    [P, 8, 2, W3], f8, tag="wo", name=f"wot{mc}_{dd}_{qq}"
                )
                nc.sync.dma_start(
                    out=wot,
                    in_=woT[:, mc, dd * ST + qq * 8 : dd * ST + (qq + 1) * 8, :, :],
                )
                wot_tiles[(mc, dd, qq)] = wot

        # phase-1 prefetch: one quarter per phase-1b s-tile (mc 0-2
        # resident by phase-1 end) — one 3.2us transfer fits each s-tile's
        # spare DMA bandwidth without starving the next s-tile's x
        WO_PREFETCH_ST = tuple(range(4, 16))
        # col D is all-ones: PV's rhs [V | 1] also accumulates the softmax
        # denominator into the A psum's column D
        nc.vector.memset(Vt[:, :, D : D + 1], 1.0)
        if n_uniq > 0:
            mbt = pers.tile([P, n_uniq, P], bf16)
        if n_uniq_add > 0:
            mbat = pers.tile([P, n_uniq_add, P], f32)

        # ---------------- phase 1: projections + rope + layout ----------------
        with (
            tc.tile_pool(name="wpool", bufs=1) as wpool,
            tc.tile_pool(name="xpool", bufs=4) as xpool,
            tc.tile_pool(name="cspool", bufs=4) as cspool,
            tc.tile_pool(name="rpool", bufs=2) as rpool,
            tc.tile_pool(name="qps", bufs=4, space="PSUM") as qps,
            tc.tile_pool(name="kvps", bufs=4, space="PSUM") as kvps,
        ):
            wTt = wpool.tile([P, DD, 2, EW + 2 * D], f8)
            wTr = wT

            XG = min(8, DD)  # dd-tiles per streamed x chunk
            xTr = xT
            # Phase 1a streams the weights ONCE while consuming them
            # pair-major across the first NA s-tiles: the startup window is
            # pure DMA capacity (w 6.3MB + x), and s-tile-major order would
            # leave the PE idle ~half of it waiting for late weight pairs.
            NA = 4
            # Per-round x chunks for the NA s-tiles with the round's weight
            # pieces riding along. DMA emission is DEFERRED and woven into
            # the matmul emission below: the xT0 ring has 6 slots for 16
            # chunks, so a chunk's dma_start must be emitted only after the
            # matmuls reading its ring-predecessor exist, or the WAR is lost
            # and the transfer overwrites live data.
            xa = [[None] * (DD // 2) for _ in range(NA)]

            def emit_xa_dma(c, s):
                xTt = xpool.tile([P, XG, 2, P], f8, tag="xT0", bufs=6)
                nc.sync.dma_start(
                    out=xTt, in_=xTr[:, s, c * XG : (c + 1) * XG, :, :]
                )
                for tt in range(0, XG, 2):
                    xa[s][(c * XG + tt) // 2] = (c * XG + tt, xTt, tt)
                if c == 0:
                    # round 0: one weight pair after each x chunk, so the
                    # first pairs' operands land just-in-time
                    w0 = 2 * s
                    nc.sync.dma_start(
                        out=wTt[:, w0 : w0 + 2, :, :],
                        in_=wTr[:, w0 : w0 + 2, :, :],
                    )
                elif s in (0, 2):
                    w0 = c * XG + (s // 2) * (XG // 2)
                    nc.sync.dma_start(
                        out=wTt[:, w0 : w0 + XG // 2, :, :],
                        in_=wTr[:, w0 : w0 + XG // 2, :, :],
                    )

            def stream_x(st):
                chunks = []
                for g in range(DD // XG):
                    xTt = xpool.tile([P, XG, 2, P], f8, tag="xT")
                    nc.sync.dma_start(
                        out=xTt,
                        in_=xTr[:, st, g * XG : (g + 1) * XG, :, :],
                    )
                    chunks.extend(
                        (g * XG + tt, xTt, tt) for tt in range(0, XG, 2)
                    )
                return chunks

            DR = mybir.MatmulPerfMode.DoubleRow

            def mm_qkv(Qp, KVp, xTt, tt, t):
                # contraction pair (t, t+1): three DoubleRow terms. The two
                # hi-weight terms of BOTH psums come before the lo-weight
                # terms, so on the in-order PE queue the w-hi half of a
                # streamed weight pair enables 4 of 6 matmuls immediately.
                x_hh = xTt[:, tt : tt + 2, 0, :]
                x_ll = xTt[:, tt : tt + 2, 1, :]
                w_hh = wTt[:, t : t + 2, 0, 0:EW]
                w_ll = wTt[:, t : t + 2, 1, 0:EW]
                v_hh = wTt[:, t : t + 2, 0, EW : EW + 2 * D]
                v_ll = wTt[:, t : t + 2, 1, EW : EW + 2 * D]
                first, last = t == 0, t == DD - 2
                nc.tensor.matmul(
                    Qp, x_hh, w_hh, start=first, stop=False, perf_mode=DR
                )
                nc.tensor.matmul(
                    Qp, x_ll, w_hh, start=False, stop=False, perf_mode=DR
                )
                nc.tensor.matmul(
                    KVp, x_hh, v_hh, start=first, stop=False, perf_mode=DR
                )
                nc.tensor.matmul(
                    KVp, x_ll, v_hh, start=False, stop=False, perf_mode=DR
                )
                nc.tensor.matmul(
                    Qp, x_hh, w_ll, start=False, stop=last, perf_mode=DR
                )
                nc.tensor.matmul(
                    KVp, x_hh, v_ll, start=False, stop=last, perf_mode=DR
                )

            def mm_kv_only(KVp, xTt, tt, t):
                x_hh = xTt[:, tt : tt + 2, 0, :]
                x_ll = xTt[:, tt : tt + 2, 1, :]
                v_hh = wTt[:, t : t + 2, 0, EW : EW + 2 * D]
                v_ll = wTt[:, t : t + 2, 1, EW : EW + 2 * D]
                first, last = t == 0, t == DD - 2
                nc.tensor.matmul(
                    KVp, x_hh, v_hh, start=first, stop=False, perf_mode=DR
                )
                nc.tensor.matmul(
                    KVp, x_ll, v_hh, start=False, stop=False, perf_mode=DR
                )
                nc.tensor.matmul(
                    KVp, x_hh, v_ll, start=False, stop=last, perf_mode=DR
                )

            def mm_q_only(Qp, xTt, tt, t):
                x_hh = xTt[:, tt : tt + 2, 0, :]
                x_ll = xTt[:, tt : tt + 2, 1, :]
                w_hh = wTt[:, t : t + 2, 0, 0:EW]
                w_ll = wTt[:, t : t + 2, 1, 0:EW]
                first, last = t == 0, t == DD - 2
                nc.tensor.matmul(
                    Qp, x_hh, w_hh, start=first, stop=False, perf_mode=DR
                )
                nc.tensor.matmul(
                    Qp, x_ll, w_hh, start=False, stop=False, perf_mode=DR
                )
                nc.tensor.matmul(
                    Qp, x_hh, w_ll, start=False, stop=last, perf_mode=DR
                )

            # rope via strided even/odd halves (2-level APs only - 3-level
            # APs overflow the fixed ISA instruction encoding).
            def ttr_ew(out, in0, in1, op):
                nc.vector.tensor_tensor(out=out, in0=in0, in1=in1, op=op)

            A_ = mybir.AluOpType
            HF = EW // 2  # 256: cos table width for q

            def rope_k(st_, KVp_, cst_):
                t1 = rpool.tile([P, D // 2], f32, tag="t1")
                t2 = rpool.tile([P, D // 2], f32, tag="t2")
                rk = rpool.tile([P, D], bf16, tag="rk")
                k_ev, k_od = KVp_[:, 0:D:2], KVp_[:, 1:D:2]
                cosk, sink = cst_[:, 0 : D // 2], cst_[:, HF : HF + D // 2]
                ttr_ew(t1, k_ev, cosk, A_.mult)
                ttr_ew(t2, k_od, sink, A_.mult)
                ttr_ew(rk[:, 0:D:2], t1, t2, A_.subtract)
                ttr_ew(t1, k_ev, sink, A_.mult)
                ttr_ew(t2, k_od, cosk, A_.mult)
                ttr_ew(rk[:, 1:D:2], t1, t2, A_.add)

                # V -> bf16 [k, d] layout (ACT copy, cast, undo the x64
                # weight scaling)
                nc.scalar.activation(
                    out=Vt[:, st_, 0:D],
                    in_=KVp_[:, D : 2 * D],
                    func=mybir.ActivationFunctionType.Copy,
                    scale=inv64,
                )
                nc.sync.dma_start_transpose(
                    out=KTt[:, st_ * P : (st_ + 1) * P], in_=rk
                )

            def rope_q(st_, Qp_, cst_):
                rq = rpool.tile([P, EW], bf16, tag="rq")
                t1 = rpool.tile([P, HF], f32, tag="t1")
                t2 = rpool.tile([P, HF], f32, tag="t2")
                cosr, sinr = cst_[:, 0:HF], cst_[:, HF : 2 * HF]
                q_ev, q_od = Qp_[:, 0:EW:2], Qp_[:, 1:EW:2]
                ttr_ew(t1, q_ev, cosr, A_.mult)
                ttr_ew(t2, q_od, sinr, A_.mult)
                ttr_ew(rq[:, 0:EW:2], t1, t2, A_.subtract)
                ttr_ew(t1, q_ev, sinr, A_.mult)
                ttr_ew(t2, q_od, cosr, A_.mult)
                ttr_ew(rq[:, 1:EW:2], t1, t2, A_.add)

                # transpose rq (per head) into [d, s] via the DMA
                # transpose engine (keeps PE free for matmuls)
                nc.sync.dma_start_transpose(
                    out=QTt[:, :, st_ * P : (st_ + 1) * P], in_=rq
                )

            # ---- phase 1a: s-tiles 0..NA-1 pair-major vs the streaming w:
            # each weight pair is consumed against NA s-tiles as it lands,
            # so the PE tracks the DMA-capacity-bound startup window instead
            # of idling for late pairs
            Qpa = [
                qps.tile([P, EW], f32, tag="Qp", name=f"Qpa{s}")
                for s in range(NA)
            ]
            KVpa = [
                kvps.tile([P, 2 * D], f32, tag="KVp", name=f"KVpa{s}")
                for s in range(NA)
            ]
            # fresh xT0 slots: round 0 + half of round 1
            for s in range(NA):
                emit_xa_dma(0, s)
            emit_xa_dma(1, 0)
            emit_xa_dma(1, 1)
            # masks are tiny; land them long before phase 2 needs them
            if n_uniq > 0:
                nc.sync.dma_start(out=mbt, in_=mb.rearrange("u p m -> p u m"))
            if n_uniq_add > 0:
                nc.sync.dma_start(out=mbat, in_=mba.rearrange("u p m -> p u m"))
            pending_dma = [(1, 2), (1, 3)] + [
                (c, s) for c in (2, 3) for s in range(NA)
            ]
            for rnd in range(DD // XG):
                for pr in range(rnd * 4, rnd * 4 + 4):
                    for s in range(NA):
                        t, xTt, tt = xa[s][pr]
                        mm_qkv(Qpa[s], KVpa[s], xTt, tt, t)
                # ring slots of the next chunks now have their readers
                # emitted; release the next DMAs
                for _ in range(4 if rnd < 2 else 2):
                    if pending_dma:
                        emit_xa_dma(*pending_dma.pop(0))
            cs_a = []
            for s in range(NA):
                csa = cspool.tile([P, EW], bf16, tag="cs")
                nc.sync.dma_start(out=csa, in_=cs[s * P : (s + 1) * P, :])
                cs_a.append(csa)
            for s in range(NA):
                rope_k(s, KVpa[s], cs_a[s])
                rope_q(s, Qpa[s], cs_a[s])

            # ---- phase 1b: remaining s-tiles, s-tile-major ----
            wo_pending = [False]

            def flush_wo_pending():
                if wo_pending[0]:
                    wo_pending[0] = False
                    load_next_wo(wopool, 1)

            for st in range(NA, ST):
                cst = cspool.tile([P, EW], bf16, tag="cs")
                nc.sync.dma_start(out=cst, in_=cs[st * P : (st + 1) * P, :])

                if st >= ST - 3:
                    # last s-tiles: run the whole KV contraction first and
                    # rope/transpose K before the Q matmuls, so the K tiles
                    # phase 2's first score chunks need are ready ~5us
                    # earlier (the K transpose rides the in-order DMA queue)
                    Qp = qps.tile([P, EW], f32, tag="Qp")
                    KVp = kvps.tile([P, 2 * D], f32, tag="KVp")
                    chunks = stream_x(st)
                    flush_wo_pending()
                    for t, xTt, tt in chunks:
                        mm_kv_only(KVp, xTt, tt, t)
                    rope_k(st, KVp, cst)
                    for t, xTt, tt in chunks:
                        mm_q_only(Qp, xTt, tt, t)
                    later = [(st, Qp, None, cst)]
                else:
                    Qp = qps.tile([P, EW], f32, tag="Qp")
                    KVp = kvps.tile([P, 2 * D], f32, tag="KVp")
                    for t, xTt, tt in stream_x(st):
                        mm_qkv(Qp, KVp, xTt, tt, t)
                    flush_wo_pending()
                    later = [(st, Qp, KVp, cst)]

                for st_, Qp_, KVp_, cst_ in later:
                    if KVp_ is not None:
                        rope_k(st_, KVp_, cst_)
                    rope_q(st_, Qp_, cst_)

                if st in WO_PREFETCH_ST:
                    # deferred: emitted after the NEXT s-tile's x chunks so
                    # the 3.2us transfer never delays them on the queue
                    wo_pending[0] = True
            flush_wo_pending()

        # ---------------- phase 2: attention (transposed scores) --------------
        apool = ctx.enter_context(tc.tile_pool(name="apool", bufs=1))
        # split by head-pair so phase 3's first row-tile can start once
        # heads 0-1 finish, overlapping the rest of phase 2. A is stored as
        # fp8 hi/lo pairs for the compensated-fp8 output projection.
        Aall = [
            apool.tile([P, 2 * ST * D], f8, name=f"Aall{i}")
            for i in range(NH // 2)
        ]
        Aallr = [
            apool.tile([P, 2 * ST * D], f8, name=f"Aallr{i}")
            for i in range(NH // 2)
        ]
        # PSUM pool order matters: pools opened first reuse phase 1's freed
        # qps/kvps bytes and inherit a WAR on the last s-tile's rope reads.
        # ops (phase 3) and aps/dsps (needed a few steps into phase 2) absorb
        # that; sps (needed immediately) lands on fresh bytes.
        with (
            tc.tile_pool(name="ops", bufs=2, space="PSUM") as ops,
            tc.tile_pool(name="aps", bufs=2, space="PSUM") as aps,
            tc.tile_pool(name="sps", bufs=2, space="PSUM") as sps,
            tc.tile_pool(name="ptsb", bufs=4) as ptsb,
            tc.tile_pool(name="stat", bufs=8) as stat,
            tc.tile_pool(name="osb", bufs=7) as osb,
            tc.tile_pool(name="wopoolB", bufs=5) as wopoolB,
        ):
            # q-tiles processed long-rows-first so the front/back pipeline
            # fills with real work instead of stalling on semaphore chains
            # through tiny rows at phase-2 entry. Head 0 starts at i=12 (not
            # 15) because q-tile 15's transpose is still draining through the
            # DMA queue when phase 2 begins.
            steps = []
            h0_order = [10, 11, 12, 13, 14, 15] + list(range(9, -1, -1))
            for h in range(NH):
                for i in (h0_order if h == 0 else reversed(range(ST))):
                    if plan[i]:
                        steps.append((h, i))

            # per-(h, qs) psum tiles holding 4 query-tiles' worth of slots;
            # accumulation groups are time-sequential so sharing one 2KB
            # zero-region is safe (earlier slots are only read afterwards)
            blk_tiles = {}

            def emit_front(step):
                """Scores (PE) + exp (ACT) + causal 0/1 multiply (DVE).

                8-block exp chunks (2-bank PSUM S tiles): ACT instruction
                count halves, and ACT's ~185ns per-instruction access
                latency is what makes phase 2 ACT-critical."""
                h, i = step
                row = plan[i]
                PTt = ptsb.tile([P, ST, P], bf16, tag="PT")
                for c0 in range(0, len(row), 8):
                    chunk = row[c0 : c0 + 8]
                    S = sps.tile([P, 1024], f32, tag="S")
                    for j, (kt, uid, uida) in enumerate(chunk):
                        nc.tensor.matmul(
                            S[:, j * P : (j + 1) * P],
                            KTt[:, kt * P : (kt + 1) * P],
                            QTt[:, h, i * P : (i + 1) * P],
                            start=True,
                            stop=True,
                        )
                        if uida >= 0:
                            nc.vector.tensor_add(
                                S[:, j * P : (j + 1) * P],
                                S[:, j * P : (j + 1) * P],
                                mbat[:, uida, :],
                            )
                    nc.scalar.activation(
                        out=PTt[:, c0 : c0 + len(chunk), :],
                        in_=S[:, 0 : len(chunk) * P],
                        func=mybir.ActivationFunctionType.Exp,
                        bias=ebias,
                    )
                    for j, (kt, uid, uida) in enumerate(chunk):
                        if uid >= 0:
                            nc.vector.tensor_tensor(
                                out=PTt[:, c0 + j, :],
                                in0=PTt[:, c0 + j, :],
                                in1=mbt[:, uid, :],
                                op=mybir.AluOpType.mult,
                            )
                return PTt

            def emit_back(step, PTt):
                """PV with fused denominator (PE) + recip (DVE) + normalized
                fp8 hi/lo Aall writes (DVE)."""
                h, i = step
                row = plan[i]
                qs, qi = i // 4, i % 4
                A = aps.tile([P, D + 1], f32, tag="A")
                nkt = len(row)
                for n, (kt, uid, uida) in enumerate(row):
                    nc.tensor.matmul(
                        A,
                        PTt[:, n, :],
                        Vt[:, kt, :],
                        start=(n == 0),
                        stop=(n == nkt - 1),
                    )
                rec = stat.tile([P, 1], f32, tag="rec")
                nc.vector.reciprocal(rec, A[:, D : D + 1])
                # Aall layout: [sp, (t*2 + dd)*128 + hb*64 + p] so the final
                # matmul's stationary slices are contiguous (walrus requires
                # a single free dim on weight APs)
                hb = h % 2

                def dv(Ah):
                    # dview[sp, p, dd] == Ah[:, i*256 + dd*128 + hb*64 + p]
                    return Ah[:, i * 2 * P : (i + 1) * 2 * P].rearrange(
                        "a (dd j) -> a dd j", dd=2
                    )[:, :, hb * 64 : hb * 64 + 64].rearrange(
                        "a dd p -> a p dd"
                    )

                dhi, dlo = dv(Aall[h // 2]), dv(Aallr[h // 2])
                Asl = A[:, 0:D].rearrange(
                    "a (p two) -> a p two", two=2
                )
                nc.vector.tensor_scalar_mul(dhi, Asl, rec)
                # lo = A*rec - hi (both fp8 rounded by the output dtype)
                nc.vector.scalar_tensor_tensor(
                    out=dlo,
                    in0=Asl,
                    scalar=rec,
                    in1=dhi,
                    op0=mybir.AluOpType.mult,
                    op1=mybir.AluOpType.subtract,
                )

            # zero Aall regions for fully-masked query rows (unreachable for
            # causal masks, but keeps the flatten well-defined). Emitted
            # before any phase-3 matmul can read them.
            for i in range(ST):
                if not plan[i]:
                    for h in range(NH):
                        for Ah in (Aall[h // 2], Aallr[h // 2]):
                            nc.vector.memset(
                                Ah[:, i * 2 * P : (i + 1) * 2 * P], 0.0
                            )

            # ---------------- phase 3 (interleaved into phase 2) -----------
            # Phase 2 is ACT(exp)-throughput-bound, leaving the PE with idle
            # slack between steps; phase-3 matmuls are drip-fed into that
            # slack as soon as their Aall inputs are final. wo quarters for
            # mc 0-2 prefetched during phase 1; the rest ride a close-
            # triggered ring (close of (mc, 1) frees mc's 4 quarters → load
            # mc+3's), so the in-order SP queue never blocks on a WAR wait.
            p3_queue = []  # (mc, it, u) units in emission order
            p3_open = {}
            p3_cnt = {}  # units emitted per psum key; drives start/stop
            pushed = set()

            # out-store DMAs of the early it=1 tail closes ride the same
            # in-order queue as the wo ring loads and add ~0.5us each to the
            # serial load chain; they are deferred (copies still run, osb
            # slots held) and flushed once the last ring load is emitted
            deferred_out = []

            # the very last block runs as four 128-wide quarter-accumulations
            # so each quarter's copy/store overlaps the next one's matmuls
            # and the post-last-matmul drain chain is a single 128-col piece
            FINAL = (MC2 - 1, 1)

            def close_p3_block(mc, it, half):
                O = p3_open.pop((mc, it, half))
                if (mc, it) == (3, 1):
                    # (3,1) closing frees wopoolB's mc3 slots + wopool's
                    # spare; later closes free wopool's mc0-2 slots in ring
                    # order, so pops always land where the WAR clears next
                    load_next_wo(wopoolB, 3)
                    load_next_wo(wopool, 1)
                elif it == 1 and half in (None, 3):
                    load_next_wo(wopool, 4)
                base = mc * W3 + (half * 128 if half is not None else 0)
                width = 128 if half is not None else W3
                npc = 1
                w = width // npc
                for pc in range(npc):
                    Ot = osb.tile([P, w], bf16, tag="Ot")
                    nc.scalar.activation(
                        out=Ot,
                        in_=O[:, pc * w : (pc + 1) * w],
                        func=mybir.ActivationFunctionType.Copy,
                        scale=inv64,
                    )
                    dst = out[
                        it * P : (it + 1) * P,
                        base + pc * w : base + (pc + 1) * w,
                    ]
                    if it == 1 and mc <= 3 and half is None:
                        deferred_out.append((dst, Ot))
                    else:
                        nc.sync.dma_start(out=dst, in_=Ot)
                if (mc, it) == (4, 0):
                    # all ring loads are emitted by close(3,1); flushing here
                    # puts these stores behind the last wo transfer
                    for dst, Ot in deferred_out:
                        nc.sync.dma_start(out=dst, in_=Ot)
                    deferred_out.clear()

            NU = 3 * JT // 2  # 48 DoubleRow units per block

            def pair_ap(Ah, ddj, t):
                idx = t * 2 + ddj
                return Ah.rearrange("a (tt j) -> a tt j", j=P)[
                    :, idx : idx + 3 : 2, :
                ]

            def push_block(mc, it):
                pushed.add((mc, it))
                n = 4 * NU if (mc, it) == FINAL else NU
                p3_queue.extend([(mc, it, u) for u in range(n)])

            def emit_p3(budget):
                emitted = 0
                while p3_queue and emitted < budget:
                    mc, it, u = p3_queue.pop(0)
                    if (mc, it) == FINAL:
                        half, uu = divmod(u, NU)
                    else:
                        half, uu = None, u
                    key = (mc, it, half)
                    if key not in p3_open:
                        p3_open[key] = ops.tile(
                            [P, 128 if half is not None else 512],
                            f32,
                            tag="O",
                            name=f"O{mc}_{it}_{half}",
                        )
                    O = p3_open[key]
                    pi, term = uu // 3, uu % 3
                    t, ddj = 2 * (pi // 2), pi % 2
                    lhsT = pair_ap(
                        (Aall if term != 1 else Aallr)[it], ddj, t
                    )
                    rhs = wot_tiles[(mc, ddj, t // 8)][
                        :, t % 8 : t % 8 + 2, 1 if term == 2 else 0, :
                    ]
                    if half is not None:
                        rhs = rhs[:, :, half * 128 : (half + 1) * 128]
                    # drip-fed units arrive out of unit-id order, so the
                    # psum group's start (zeroing) / stop must track the
                    # EMISSION count, not the unit id
                    cnt = p3_cnt.get(key, 0)
                    p3_cnt[key] = cnt + 1
                    nc.tensor.matmul(
                        O,
                        lhsT,
                        rhs,
                        start=(cnt == 0),
                        stop=(cnt == NU - 1),
                        perf_mode=DR,
                    )
                    emitted += 1
                    if cnt == NU - 1:
                        close_p3_block(mc, it, half)
                return emitted

            # stage the next 5 quarters (3_0_0..4_0_0) into the spare wopool
            # slot + the fresh wopoolB ring right at phase-2 start: fresh
            # slots have no WAR, so these transfers run in phase 2's
            # otherwise-idle DMA window
            load_next_wo(wopool, 1)
            load_next_wo(wopoolB, 5)

            # Deep software pipeline: PE runs step n's scores while ACT/DVE
            # finish earlier steps, so the PE never waits on exp results
            DEPTH = 3
            pending = []

            all_rows = all(plan[i] for i in range(ST))

            steps_h1 = [s for s in steps if s[0] == 1]

            def after_back(s0, front_step):
                h0_, i0_ = s0
                if steps_h1 and s0 == steps_h1[-1]:
                    # ALL of head 0-1's PV writes are now emitted: (1,0)'s
                    # units may be queued without reading not-yet-written
                    # Aall[0] rows (pushing at head-2's first FRONT would
                    # race the last DEPTH backs of head 1)
                    if not all_rows and (0, 0) not in pushed:
                        push_block(0, 0)
                    push_block(1, 0)
                if h0_ == 1 and all_rows and i0_ % 2 == 0:
                    # with descending q-tile order, rows (i0, i0+1) are both
                    # final once head 1 reaches even i0; drip block (0,0)'s
                    # matching jt-pair units in right here
                    pushed.add((0, 0))
                    for pi in (i0_, i0_ + 1):
                        p3_queue.extend(
                            [(0, 0, 3 * pi + tm) for tm in range(3)]
                        )
                if h0_ == 3 and all_rows and i0_ % 2 == 0:
                    # same for (0,1) as head 3 completes its rows: closing
                    # (0,1) at the very start of the tail frees mc0's wopool
                    # slots, so the mc4-7 load chain starts ~5us earlier
                    pushed.add((0, 1))
                    for pi in (i0_, i0_ + 1):
                        p3_queue.extend(
                            [(0, 1, 3 * pi + tm) for tm in range(3)]
                        )
                # budget ~ the ACT-over-PE slack of the step the PE is
                # currently chewing on
                nch = (len(plan[front_step[1]]) + 3) // 4 if front_step else 2
                emit_p3(max(3, min(9, 2 * nch + 3)))

            # blocks (1,0) and (2,0) become ready when heads 0-1 are done
            # (mc 0-2 wo quarters are phase-1-prefetched; mc 3+ stay in the
            # tail where the close-triggered ring covers them)
            steps_h2 = [s for s in steps if s[0] == 2]
            steps_h3 = [s for s in steps if s[0] == 3]
            for step in steps:
                if steps_h3 and step == steps_h3[0]:
                    push_block(2, 0)
                    push_block(3, 0)
                PTt = emit_front(step)
                pending.append((step, PTt))
                if len(pending) > DEPTH:
                    s0, p0 = pending.pop(0)
                    emit_back(s0, p0)
                    after_back(s0, step)
            for s0, p0 in pending:
                emit_back(s0, p0)
                after_back(s0, None)

            # remaining blocks: it=1 of mc0-3 first — their closes free the
            # wopool/wopoolB slots for mc4-7's ring loads in consumption
            # order ((0,1) leftovers drain first from the head-3 drip)
            base_rest = [(0, 1), (1, 1), (2, 1), (3, 1), (4, 0), (4, 1),
                         (5, 0), (5, 1), (6, 0), (6, 1), (7, 0), (7, 1),
                         (3, 0)]
            for mc, it in base_rest:
                if (mc, it) not in pushed:
                    push_block(mc, it)
            for mc in range(MC2):
                for it in range(ITILES):
                    if (mc, it) not in pushed:
                        push_block(mc, it)
            emit_p3(10 ** 9)

    nc.compile()
    return nc


def analyze_mask(mask, SEQ):
    """Classify 128x128 mask blocks: skip / free / masked.

    Masked blocks that only contain {0, -inf-ish} become 0/1 multiplicative
    blocks applied to exp'd scores (transposed, bf16). Blocks with other
    finite values become additive f32 blocks applied pre-exp (transposed).
    Returns (plan, mult_blocks, add_blocks); plan[i] is a list of
    (kt, uid_mult, uid_add).
    """
    ST = SEQ // P
    uniq_m, blocks_m = {}, []
    uniq_a, blocks_a = {}, []
    plan = []
    for i in range(ST):
        row = []
        for kt in range(ST):
            blk = mask[i * P : (i + 1) * P, kt * P : (kt + 1) * P]
            if (blk <= NEG_THRESH).all():
                continue
            if not blk.any():
                row.append((kt, -1, -1))
            elif ((blk == 0) | (blk <= NEG_THRESH)).all():
                key = blk.tobytes()
                if key not in uniq_m:
                    uniq_m[key] = len(blocks_m)
                    blocks_m.append(
                        np.ascontiguousarray((blk.T > NEG_THRESH).astype(np.float32))
                    )
                row.append((kt, uniq_m[key], -1))
            else:
                key = blk.tobytes()
                if key not in uniq_a:
                    uniq_a[key] = len(blocks_a)
                    blocks_a.append(np.ascontiguousarray(blk.T))
                row.append((kt, -1, uniq_a[key]))
        # fully masked query rows: leave empty; Aall is zero-filled for them
        plan.append(row)
    return plan, blocks_m, blocks_a


def make_rope_tables(cos_freq, sin_freq, SEQ, scale_quarter):
    """Build replicated [cos_rep (SEQ, NH*64) | sin_rep (SEQ, NH*64)] with
    sqrt(SCALE) folded in."""
    cos_t = np.tile(np.asarray(cos_freq, np.float32) * scale_quarter, (1, NH))
    sin_t = np.tile(np.asarray(sin_freq, np.float32) * scale_quarter, (1, NH))
    import ml_dtypes

    return np.ascontiguousarray(
        np.concatenate([cos_t, sin_t], axis=1).astype(ml_dtypes.bfloat16)
    )


_BUILD_CACHE = {}


def kernel(
    x,
    cos_freq,
    sin_freq,
    positions,
    mask,
    wq,
    wk,
    wv,
    wo,
    _trace=False,
):
    import sys

    if "/opt/trn_rl_repo" not in sys.path:
        sys.path.insert(0, "/opt/trn_rl_repo")
    from concourse.bass_utils import run_bass_kernel_spmd

    x = np.asarray(x, np.float32)
    mask = np.asarray(mask, np.float32)
    wq = np.asarray(wq, np.float32)
    wk = np.asarray(wk, np.float32)
    wv = np.asarray(wv, np.float32)
    wo = np.asarray(wo, np.float32)
    SEQ, DIM = x.shape
    assert wq.shape[0] == CORES * NH * D and wk.shape[0] == CORES * D
    assert 2 * SEQ == wq.shape[0], "flatten structure requires H*D == 2*SEQ"

    plan, blocks_m, blocks_a = analyze_mask(mask, SEQ)
    n_uniq, n_uniq_add = len(blocks_m), len(blocks_a)
    key = (SEQ, DIM, tuple(tuple(r) for r in plan))
    if key not in _BUILD_CACHE:
        _BUILD_CACHE[key] = build_attention_nc(SEQ, DIM, plan, n_uniq, n_uniq_add)
    nc = _BUILD_CACHE[key]

    import ml_dtypes

    bf16 = ml_dtypes.bfloat16
    f8 = ml_dtypes.float8_e4m3
    WSC = np.float32(64.0)  # weight pre-scale; undone via rope tables/V copy

    def f8hl(a):
        hi = a.astype(f8)
        lo = (a - hi.astype(np.float32)).astype(f8)
        return hi, lo

    # fold 1/64 into the rope tables (q and k both carry the x64 weights)
    scale_quarter = np.float32(D ** -0.25) / WSC
    cs = make_rope_tables(cos_freq, sin_freq, SEQ, scale_quarter)
    ST_, DD_ = SEQ // P, DIM // P
    xt = np.ascontiguousarray(x.reshape(ST_, P, DD_, P).transpose(3, 0, 2, 1))
    xh, xl = f8hl(xt)
    xT = np.ascontiguousarray(np.stack([xh, xl], axis=3))  # [p, st, t, 2, si]
    wot3 = np.ascontiguousarray(
        (WSC * wo.T).reshape(2 * SEQ // P, P, DIM).transpose(1, 0, 2)
    )  # [p, jt, m] = 64 * wo[m, jt*128+p]
    woh, wol = f8hl(wot3)
    JT_ = 2 * SEQ // P
    woT = np.ascontiguousarray(
        np.stack([woh, wol], axis=2)
        .reshape(P, JT_, 2, DIM // 512, 512)
        .transpose(0, 3, 1, 2, 4)
    )  # [p, mc, jt, hl, mi]
    if n_uniq:
        mbs = np.ascontiguousarray(np.stack(blocks_m, axis=0)).astype(bf16)
    else:
        mbs = np.zeros((1, P, P), bf16)
    if n_uniq_add:
        mbas = np.ascontiguousarray(np.stack(blocks_a, axis=0)).astype(np.float32)
    else:
        mbas = np.zeros((1, P, P), np.float32)

    in_maps = []
    for c in range(CORES):
        w_c = np.concatenate(
            [
                wq[c * NH * D : (c + 1) * NH * D],
                wk[c * D : (c + 1) * D],
                wv[c * D : (c + 1) * D],
            ],
            axis=0,
        )
        wt = np.ascontiguousarray(
            (WSC * w_c.T).reshape(DD_, P, -1).transpose(1, 0, 2)
        )  # [p, t, e] = 64 * w_c[e, t*128+p]
        wh, wl = f8hl(wt)
        whl = np.ascontiguousarray(np.stack([wh, wl], axis=2))
        in_maps.append(
            {
                "xT": xT,
                "wT": whl,
                "cs": cs,
                "maskb": mbs,
                "maskba": mbas,
                "woT": woT,
            }
        )

    import time as _time

    _t0 = _time.time()
    res = run_bass_kernel_spmd(nc, in_maps, list(range(CORES)), trace=_trace)
    global LAST_EXEC_NS
    LAST_EXEC_NS = int((_time.time() - _t0) * 1e9)
    outp = np.concatenate(
        [res.results[c]["out"] for c in range(CORES)], axis=0
    ).astype(np.float32)
    if _trace:
        return outp, res
    return outp

